# revision 1
# baseline (speedup 1.0000x reference)
"""Trainium2 Bass kernel for nn_MixedTransformer (GNN encode-process-decode).

Distribution: 8 cores = 2 batch groups x 4 dst-range quarters.
Per core: dense val-table matmul, edge gathers via dma_gather, segment-softmax
message passing via one-hot matmuls into PSUM, GAT processor with table
all-gathers inside each 4-core group, decoder back to the grid.

Self-contained: hardcodes all shapes; host does edge sorting/packing and the
encoder's softmax weights (all inputs to that stage are host-visible).
"""
import sys

try:
    import concourse  # noqa: F401
except ImportError:
    sys.path.insert(0, "/opt/trn_rl_repo")

import numpy as np

# ---------------- problem constants ----------------
P = 128
BS = 2
ERA, HMESH = 35718, 10242
IN, AUX, POS = 96, 2, 4
HID, HEADS, DH = 256, 2, 128
E_E2H, E_H2H, E_H2E = 107154, 61440, 107154

ERA_PAD, NBE = 35840, 280          # padded grid rows / dst blocks
MH_PAD, NBM = 10752, 84            # padded mesh rows / dst blocks
QBM, QBE = 21, 70                  # dst blocks per quarter (mesh / grid)
HALF_A = 17920                     # stage-A source table split (int16 limit)

TA_W = 256                         # T_A row: val(256)
TB_W = 320                         # T_l row: q(256) uS(2) uD(2) pad(60)
TC_W = 128                         # T_C row: val(96) uS(1) pad(31)

RG = [[0, 1, 2, 3], [4, 5, 6, 7]]

F32 = None  # set after mybir import


# ---------------- host-side packing ----------------

def _seg_softmax_host(logits, seg, n):
    """Exact reference segment softmax (f64), returns per-edge alpha."""
    lg = logits.astype(np.float64)
    m = np.full(n, -np.inf)
    np.maximum.at(m, seg, lg)
    e = np.exp(lg - m[seg])
    s = np.zeros(n)
    np.add.at(s, seg, e)
    return (e / (s[seg] + 1e-9)).astype(np.float64)


def _block_partition(src, dst, nblocks, qb, split_half=None):
    """Group edges by 128-row dst block; per program slot s (0..qb-1) compute
    uniform tile counts K (max over the 4 quarters); return structure."""
    blk = dst // P
    order = np.argsort(blk, kind="stable")
    bo = blk[order]
    starts = np.searchsorted(bo, np.arange(nblocks + 1))
    per_block = [order[starts[j]:starts[j + 1]] for j in range(nblocks)]
    if split_half is not None:
        per_block_lo, per_block_hi = [], []
        for j in range(nblocks):
            e = per_block[j]
            lo = e[src[e] < split_half]
            hi = e[src[e] >= split_half]
            per_block_lo.append(lo)
            per_block_hi.append(hi)
        K_lo = [max(-(-len(per_block_lo[qb * r + s]) // P) for r in range(4))
                for s in range(qb)]
        K_hi = [max(-(-len(per_block_hi[qb * r + s]) // P) for r in range(4))
                for s in range(qb)]
        return per_block_lo, per_block_hi, K_lo, K_hi
    K = [max(-(-len(per_block[qb * r + s]) // P) for r in range(4))
         for s in range(qb)]
    return per_block, K


def _wrap_idx16(idx_flat):
    """Pack int indices for dma_gather: idx j -> [j%16, j//16], tiled to 128
    partitions. idx_flat length must be a multiple of 128."""
    n = len(idx_flat)
    cols = n // 16
    arr = np.zeros((16, cols), np.int16)
    arr[np.arange(n) % 16, np.arange(n) // 16] = idx_flat
    return np.tile(arr, (8, 1))


def _pad_to(arr, n, fill):
    out = np.full(n, fill, arr.dtype)
    out[:len(arr)] = arr
    return out


class _Packed:
    pass


def _host_prep(inputs):
    f32 = np.float32
    x = np.asarray(inputs["x"], f32)
    e2h = np.asarray(inputs["e2h_idx"]).astype(np.int64)
    h2h = np.asarray(inputs["h2h_idx"]).astype(np.int64)
    h2e = np.asarray(inputs["h2e_idx"]).astype(np.int64)
    e2h_attr = np.asarray(inputs["e2h_attr"], f32)
    h2h_attr = np.asarray(inputs["h2h_attr"], f32)
    h2e_attr = np.asarray(inputs["h2e_attr"], f32)
    era_ll = np.asarray(inputs["era_latlons"], f32)
    h_ll = np.asarray(inputs["h_latlons"], f32)
    fm_ctx = np.asarray(inputs["fm_ctx"], f32)
    fm_Wsrc = np.asarray(inputs["fm_Wsrc"], f32)
    fm_Wctx = np.asarray(inputs["fm_Wctx"], f32)
    fm_Wedge = np.asarray(inputs["fm_Wedge"], f32)
    fm_att = np.asarray(inputs["fm_att"], f32)
    fm_Wval = np.asarray(inputs["fm_Wval"], f32)
    bm_ctx = np.asarray(inputs["bm_ctx"], f32)
    bm_Wsrc = np.asarray(inputs["bm_Wsrc"], f32)
    bm_Wctx = np.asarray(inputs["bm_Wctx"], f32)
    bm_Wedge = np.asarray(inputs["bm_Wedge"], f32)
    bm_att = np.asarray(inputs["bm_att"], f32)
    bm_Wval = np.asarray(inputs["bm_Wval"], f32)
    gat_W = np.asarray(inputs["gat_W"], f32)
    gat_We = np.asarray(inputs["gat_We"], f32)
    gat_asrc = np.asarray(inputs["gat_asrc"], f32)
    gat_adst = np.asarray(inputs["gat_adst"], f32)
    gat_aedge = np.asarray(inputs["gat_aedge"], f32)

    pk = _Packed()

    # ---- encoder (stage A): host computes exact per-edge alpha ----
    sA, dA = e2h[0], e2h[1]
    x_in = [np.concatenate([x[g].reshape(ERA, IN + AUX), era_ll], 1)
            for g in range(BS)]                                   # (35718,102)
    fm_w_att = fm_Wsrc @ fm_att                                   # (102,)
    uC_A = np.concatenate([fm_ctx, h_ll], 1) @ (fm_Wctx @ fm_att)  # (HMESH,)
    uE_A = e2h_attr @ (fm_Wedge @ fm_att)                         # (E,)
    alphas_A = []
    for g in range(BS):
        uS = x_in[g] @ fm_w_att                                   # (ERA,)
        logit = uS[sA] + uC_A[dA] + uE_A
        lrelu = np.where(logit >= 0, logit, 0.2 * logit)
        alphas_A.append(_seg_softmax_host(lrelu, dA, HMESH))

    pbA_lo, pbA_hi, KA_lo, KA_hi = _block_partition(
        sA, dA, NBM, QBM, split_half=HALF_A)

    # ---- processor (stage B) ----
    sB, dB = h2h[0], h2h[1]
    pbB, KB = _block_partition(sB, dB, NBM, QBM)
    uE_B = [h2h_attr @ np.einsum("fhd,hd->fh", gat_We[l], gat_aedge[l])
            for l in range(2)]                                    # (E,2)
    w_s = [np.einsum("fhd,hd->fh", gat_W[l], gat_asrc[l]) for l in range(2)]
    w_d = [np.einsum("fhd,hd->fh", gat_W[l], gat_adst[l]) for l in range(2)]
    pk.w_tb = [np.concatenate(
        [gat_W[l].reshape(HID, HID), w_s[l], w_d[l]], 1) for l in range(2)]  # (256,260)

    # ---- decoder (stage C) ----
    sC, dC = h2e[0], h2e[1]
    pbC, KC = _block_partition(sC, dC, NBE, QBE)
    bm_w_att = bm_Wsrc @ bm_att                                   # (260,)
    uC_C = np.concatenate([bm_ctx, era_ll], 1) @ (bm_Wctx @ bm_att)  # (ERA,)
    uE_C = h2e_attr @ (bm_Wedge @ bm_att)                         # (E,)
    uCE_C = uC_C[dC] + uE_C

    pk.w_tc = np.concatenate([bm_Wval[:HID], bm_w_att[:HID, None]], 1)  # (256,97)
    hl_term = h_ll @ np.concatenate(
        [bm_Wval[HID:], bm_w_att[HID:, None]], 1)                 # (HMESH,97)
    hl_pad = np.zeros((MH_PAD, IN + 1), f32)
    hl_pad[:HMESH] = hl_term

    # ---- dense encoder input, transposed + tiled ----
    pk.xinT = []
    for g in range(BS):
        xt = np.zeros((IN + AUX + POS, ERA_PAD), f32)
        xt[:, :ERA] = x_in[g].T
        pk.xinT.append(np.ascontiguousarray(
            xt.reshape(IN + AUX + POS, NBE, P).transpose(1, 0, 2)))  # (280,102,128)
    pk.w_ta = fm_Wval                                             # (102,256)

    # ---- per-(quarter) edge packing (identical structure for both batches)
    pk.KA_lo, pk.KA_hi, pk.KB, pk.KC = KA_lo, KA_hi, KB, KC
    SKA = sum(KA_lo) + sum(KA_hi)
    SKB = sum(KB)
    SKC = sum(KC)
    pk.SKA, pk.SKB, pk.SKC = SKA, SKB, SKC

    def pack_quarter_A(r, g):
        sidx_lo, sidx_hi, cidx, alph = [], [], [], []
        for s in range(QBM):
            j = QBM * r + s
            elo, ehi = pbA_lo[j], pbA_hi[j]
            nlo, nhi = KA_lo[s] * P, KA_hi[s] * P
            sidx_lo.append(_pad_to(sA[elo].astype(np.int16), nlo, 0))
            sidx_hi.append(_pad_to((sA[ehi] - HALF_A).astype(np.int16), nhi, 0))
            cl = _pad_to((dA[elo] - j * P).astype(f32), nlo, -1.0)
            ch = _pad_to((dA[ehi] - j * P).astype(f32), nhi, -1.0)
            al = _pad_to(alphas_A[g][elo].astype(f32), nlo, 0.0)
            ah = _pad_to(alphas_A[g][ehi].astype(f32), nhi, 0.0)
            cidx.append(np.concatenate([cl, ch]))
            alph.append(np.concatenate([al, ah]))
        out = _Packed()
        out.sidx_lo = _wrap_idx16(np.concatenate(sidx_lo)) if sum(KA_lo) else np.zeros((P, 1), np.int16)
        out.sidx_hi = _wrap_idx16(np.concatenate(sidx_hi)) if sum(KA_hi) else np.zeros((P, 1), np.int16)
        # per-tile column layout: edge i of a block -> [i%128, tilebase + i//128]
        cf = np.concatenate(cidx).reshape(SKA, P).T.copy()        # (128, SKA)
        af = np.concatenate(alph).reshape(SKA, P).T.copy()
        out.cidx, out.alpha = cf, af
        return out

    def pack_quarter_BC(r, per_block, K, qb, src, dst, streams):
        """streams: list of per-edge arrays (E,) or (E,m) -> packed (128, SK*m)."""
        SK = sum(K)
        sidx, cidx, st_out = [], [], [[] for _ in streams]
        for s in range(qb):
            j = qb * r + s
            e = per_block[j]
            n = K[s] * P
            sidx.append(_pad_to(src[e].astype(np.int16), n, 0))
            cidx.append(_pad_to((dst[e] - j * P).astype(f32), n, -1.0))
            for q, arr in enumerate(streams):
                a = arr[e]
                if a.ndim == 1:
                    a = a[:, None]
                m = a.shape[1]
                buf = np.zeros((n, m), f32)
                buf[:len(e)] = a
                st_out[q].append(buf)
        out = _Packed()
        out.sidx = _wrap_idx16(np.concatenate(sidx)) if SK else np.zeros((P, 1), np.int16)
        out.cidx = np.concatenate(cidx).reshape(SK, P).T.copy()
        out.streams = []
        for q, parts in enumerate(st_out):
            a = np.concatenate(parts, 0)                          # (SK*P, m)
            m = a.shape[1]
            out.streams.append(
                a.reshape(SK, P, m).transpose(1, 0, 2).reshape(P, SK * m).copy())
        return out

    pk.cores = []
    for c in range(8):
        g, r = c // 4, c % 4
        pc = _Packed()
        pc.A = pack_quarter_A(r, g)
        pc.B = pack_quarter_BC(r, pbB, KB, QBM, sB, dB,
                               [uE_B[0], uE_B[1]])
        pc.C = pack_quarter_BC(r, pbC, KC, QBE, sC, dC, [uCE_C])
        pc.hl = hl_pad[2688 * r:2688 * (r + 1)]
        pc.xinT = pk.xinT[g]
        pk.cores.append(pc)
    return pk


# ---------------- device program ----------------

def _build(pk):
    import concourse.bass as bass
    import concourse.mybir as mybir
    import concourse.tile as tile
    from concourse import bacc
    from concourse.masks import make_identity

    f32 = mybir.dt.float32
    i16 = mybir.dt.int16
    AO = mybir.AluOpType
    AF = mybir.ActivationFunctionType

    nc = bacc.Bacc("TRN2", target_bir_lowering=False, debug=False,
                   num_devices=8)

    # ---- external I/O ----
    SKA, SKB, SKC = pk.SKA, pk.SKB, pk.SKC
    ein = {}

    def xin(name, shape, dt=f32):
        ein[name] = nc.dram_tensor(name, shape, dt, kind="ExternalInput")
        return ein[name]

    xinT = xin("xinT", [NBE, IN + AUX + POS, P])
    w_ta = xin("w_ta", [IN + AUX + POS, TA_W])
    w_tb0 = xin("w_tb0", [HID, 260])
    w_tb1 = xin("w_tb1", [HID, 260])
    w_tc = xin("w_tc", [HID, IN + 1])
    hl = xin("hl", [QBM * P, IN + 1])
    a_slo = xin("a_slo", [P, max(sum(pk.KA_lo), 1) * 8], i16)
    a_shi = xin("a_shi", [P, max(sum(pk.KA_hi), 1) * 8], i16)
    a_cidx = xin("a_cidx", [P, SKA])
    a_alpha = xin("a_alpha", [P, SKA])
    b_sidx = xin("b_sidx", [P, SKB * 8], i16)
    b_cidx = xin("b_cidx", [P, SKB])
    b_ue0 = xin("b_ue0", [P, SKB * 2])
    b_ue1 = xin("b_ue1", [P, SKB * 2])
    c_sidx = xin("c_sidx", [P, SKC * 8], i16)
    c_cidx = xin("c_cidx", [P, SKC])
    c_uce = xin("c_uce", [P, SKC])
    out_t = nc.dram_tensor("out", [QBE * P, IN], f32, kind="ExternalOutput")
    import os
    _dbg = bool(int(os.environ.get("KERNEL_DEBUG", "0")))
    _lvl = int(os.environ.get("KERNEL_PHASES", "8"))
    if _dbg:
        dbg_xlat = nc.dram_tensor("dbg_xlat", [P, QBM * HID], f32,
                                  kind="ExternalOutput")
        dbg_h1g = nc.dram_tensor("dbg_h1g", [P, QBM * HID], f32,
                                 kind="ExternalOutput")
        dbg_xproc = nc.dram_tensor("dbg_xproc", [P, QBM * HID], f32,
                                   kind="ExternalOutput")
        dbg_tb1 = nc.dram_tensor("dbg_tb1", [MH_PAD, TB_W], f32,
                                 kind="ExternalOutput")
        K0 = max(pk.KB[0], 1)
        dbg_eu = nc.dram_tensor("dbg_eu", [P, 2 * K0], f32,
                                kind="ExternalOutput")
        dbg_ud = nc.dram_tensor("dbg_ud", [P, 2 * K0], f32,
                                kind="ExternalOutput")
        dbg_udblk = nc.dram_tensor("dbg_udblk", [P, 2], f32,
                                   kind="ExternalOutput")
        dbg_us = nc.dram_tensor("dbg_us", [P, 2 * K0], f32,
                                kind="ExternalOutput")
        dbg_ps = nc.dram_tensor("dbg_ps", [P, HID + 2], f32,
                                kind="ExternalOutput")
        dbg_vs0 = nc.dram_tensor("dbg_vs0", [P, HID], f32,
                                 kind="ExternalOutput")
        dbg_ob0 = nc.dram_tensor("dbg_ob0", [P, P], f32,
                                 kind="ExternalOutput")

    KA_lo, KA_hi, KB, KC = pk.KA_lo, pk.KA_hi, pk.KB, pk.KC
    KT_A = [KA_lo[s] + KA_hi[s] for s in range(QBM)]
    GMAX = max(max(KT_A) * TA_W, max(KB) * TB_W, max(KC) * TC_W)

    with tile.TileContext(nc) as tc:
        with tc.tile_pool(name="const", bufs=1) as cpool, \
             tc.tile_pool(name="stream", bufs=1) as spool, \
             tc.tile_pool(name="res", bufs=1) as rpool, \
             tc.tile_pool(name="gat", bufs=2) as gpool, \
             tc.tile_pool(name="work", bufs=3) as wpool, \
             tc.tile_pool(name="ob", bufs=2) as obpool, \
             tc.tile_pool(name="psA", bufs=2, space="PSUM") as psA, \
             tc.tile_pool(name="psU", bufs=2, space="PSUM") as psU, \
             tc.tile_pool(name="psT", bufs=2, space="PSUM") as psT, \
             tc.tile_pool(name="dram", bufs=1, space="DRAM") as dpool:

            # ---------- constants / streams ----------
            ident = cpool.tile([P, P], f32, name="ident")
            make_identity(nc, ident[:])
            iota_i = cpool.tile([P, P], mybir.dt.int32, name="iota_i")
            nc.gpsimd.iota(iota_i[:], pattern=[[1, P]], base=0,
                           channel_multiplier=0)
            iota_f = cpool.tile([P, P], f32, name="iota_f")
            nc.vector.tensor_copy(iota_f[:], iota_i[:])

            def load(name, src, shape, dt=f32):
                t = spool.tile(shape, dt, name=name)
                nc.sync.dma_start(out=t[:], in_=src[tuple(slice(0, s) for s in shape)])
                return t

            w_ta_sb = load("w_ta_sb", w_ta, [IN + AUX + POS, TA_W])

            def load_half(name, src, h, cols):
                t = spool.tile([P, cols], f32, name=name)
                nc.sync.dma_start(out=t[:], in_=src[h * P:(h + 1) * P, 0:cols])
                return t[:]

            w_tb_sb = [[load_half(f"w_tb{l}_{h}", [w_tb0, w_tb1][l], h, 260)
                        for h in range(2)] for l in range(2)]
            w_tc_sb = [load_half(f"w_tc_{h}", w_tc, h, IN + 1)
                       for h in range(2)]

            slo_sb = load("slo_sb", a_slo, [P, max(sum(KA_lo), 1) * 8], i16)
            shi_sb = load("shi_sb", a_shi, [P, max(sum(KA_hi), 1) * 8], i16)
            acid_sb = load("acid_sb", a_cidx, [P, SKA])
            aal_sb = load("aal_sb", a_alpha, [P, SKA])
            bsid_sb = load("bsid_sb", b_sidx, [P, SKB * 8], i16)
            bcid_sb = load("bcid_sb", b_cidx, [P, SKB])
            bue_sb = [load("bue0_sb", b_ue0, [P, SKB * 2]),
                      load("bue1_sb", b_ue1, [P, SKB * 2])]
            csid_sb = load("csid_sb", c_sidx, [P, SKC * 8], i16)
            ccid_sb = load("ccid_sb", c_cidx, [P, SKC])
            cuce_sb = load("cuce_sb", c_uce, [P, SKC])

            # ---------- resident quarter features ----------
            xlat = rpool.tile([P, QBM * HID], f32, name="xlat")
            h1g = rpool.tile([P, QBM * HID], f32, name="h1g")
            xproc = rpool.tile([P, QBM * HID], f32, name="xproc")
            nc.vector.memset(xlat[:], 0.0)
            nc.vector.memset(h1g[:], 0.0)
            nc.vector.memset(xproc[:], 0.0)

            # ---------- DRAM tables ----------
            ta_dram = dpool.tile([ERA_PAD, TA_W], f32, name="ta_dram")
            tb_loc = [dpool.tile([QBM * P, TB_W], f32, name=f"tb_loc{l}")
                      for l in range(2)]
            tb_full = [dpool.tile([MH_PAD, TB_W], f32, name=f"tb_full{l}")
                       for l in range(2)]
            tc_loc = dpool.tile([QBM * P, TC_W], f32, name="tc_loc")
            tc_full = dpool.tile([MH_PAD, TC_W], f32, name="tc_full")

            # ---------- phase 1: dense T_A ----------
            for j in range(NBE if _lvl >= 1 else 0):
                lx = wpool.tile([IN + AUX + POS, P], f32, name="lx", tag="lx")
                nc.sync.dma_start(out=lx[:], in_=xinT[j, :, :])
                pst = psT.tile([P, TA_W], f32, name="ps_ta", tag="pst")
                nc.tensor.matmul(out=pst[:], lhsT=lx[:], rhs=w_ta_sb[:],
                                 start=True, stop=True)
                sb = wpool.tile([P, TA_W], f32, name="ta_sb", tag="ta_sb")
                nc.vector.tensor_copy(sb[:], pst[:])
                nc.sync.dma_start(out=ta_dram[j * P:(j + 1) * P, :], in_=sb[:])

            # ---------- helper: one-hot ----------
            def onehot(dst_ap, cidx_col):
                nc.vector.tensor_tensor(
                    out=dst_ap, in0=cidx_col.to_broadcast([P, P]),
                    in1=iota_f[:], op=AO.is_equal)

            # ---------- phase 2: stage A (encoder edges) ----------
            ofs_lo = np.cumsum([0] + KA_lo)
            ofs_hi = np.cumsum([0] + KA_hi)
            ofs_t = np.cumsum([0] + KT_A)
            for s in range(QBM if _lvl >= 2 else 0):
                KL, KH = KA_lo[s], KA_hi[s]
                KT = KL + KH
                if KT == 0:
                    continue
                gb = gpool.tile([P, GMAX], f32, name="gbA", tag="gb")
                if KL:
                    nc.gpsimd.dma_gather(
                        out_ap=gb[:, 0:KL * TA_W].rearrange(
                            "p (k w) -> p k w", w=TA_W),
                        in_ap=ta_dram[0:HALF_A, :],
                        idxs_ap=slo_sb[:, ofs_lo[s] * 8:(ofs_lo[s] + KL) * 8],
                        num_idxs=KL * P, num_idxs_reg=KL * P, elem_size=TA_W)
                if KH:
                    nc.gpsimd.dma_gather(
                        out_ap=gb[:, KL * TA_W:KT * TA_W].rearrange(
                            "p (k w) -> p k w", w=TA_W),
                        in_ap=ta_dram[HALF_A:ERA_PAD, :],
                        idxs_ap=shi_sb[:, ofs_hi[s] * 8:(ofs_hi[s] + KH) * 8],
                        num_idxs=KH * P, num_idxs_reg=KH * P, elem_size=TA_W)
                ps = psA.tile([P, HID], f32, name="psA_t", tag="psA")
                t0 = ofs_t[s]
                for k in range(KT):
                    O = wpool.tile([P, P], f32, name="O_A", tag="oh")
                    onehot(O[:], acid_sb[:, t0 + k:t0 + k + 1])
                    S = wpool.tile([P, P], f32, name="S_A", tag="sh")
                    nc.vector.tensor_scalar_mul(
                        S[:], O[:], aal_sb[:, t0 + k:t0 + k + 1])
                    nc.tensor.matmul(
                        out=ps[:], lhsT=S[:],
                        rhs=gb[:, k * TA_W:k * TA_W + HID],
                        start=(k == 0), stop=(k == KT - 1))
                nc.vector.tensor_copy(xlat[:, s * HID:(s + 1) * HID], ps[:])

            # ---------- helper: fold resident -> table ----------
            def fold(src, wtiles, wcols, dst_dram, bias_dram=None):
                for s in range(QBM):
                    pst = psT.tile([P, HID], f32, name="ps_tr", tag="pst")
                    for h in range(2):
                        nc.tensor.transpose(
                            out=pst[:, h * P:(h + 1) * P],
                            in_=src[:, s * HID + h * P:s * HID + (h + 1) * P],
                            identity=ident[:])
                    xt = wpool.tile([P, HID], f32, name="xt", tag="xt")
                    nc.vector.tensor_copy(xt[:], pst[:])
                    psf = psT.tile([P, wcols], f32, name="ps_f", tag="pst")
                    for h in range(2):
                        nc.tensor.matmul(out=psf[:], lhsT=xt[:, h * P:(h + 1) * P],
                                         rhs=wtiles[h], start=(h == 0),
                                         stop=(h == 1))
                    fsb = wpool.tile([P, wcols], f32, name="fsb", tag="fsb")
                    if bias_dram is not None:
                        hb = wpool.tile([P, wcols], f32, name="hb", tag="hb")
                        nc.sync.dma_start(
                            out=hb[:], in_=bias_dram[s * P:(s + 1) * P, :])
                        nc.vector.tensor_tensor(out=fsb[:], in0=psf[:],
                                                in1=hb[:], op=AO.add)
                    else:
                        nc.vector.tensor_copy(fsb[:], psf[:])
                    nc.sync.dma_start(
                        out=dst_dram[s * P:(s + 1) * P, 0:wcols], in_=fsb[:])

            if _lvl >= 3:
                fold(xlat, w_tb_sb[0], 260, tb_loc[0])
                nc.gpsimd.collective_compute(
                    "AllGather", AO.bypass, replica_groups=RG,
                    ins=[tb_loc[0].opt()], outs=[tb_full[0].opt()])

            # ---------- phase 3/4: GAT layers ----------
            ofs_b = np.cumsum([0] + KB)

            def gat_layer(l, dst_res, residual):
                tfull = tb_full[l]
                for s in range(QBM):
                    K = KB[s]
                    if K == 0:
                        continue
                    gb = gpool.tile([P, GMAX], f32, name="gbB", tag="gb")
                    nc.gpsimd.dma_gather(
                        out_ap=gb[:, 0:K * TB_W].rearrange(
                            "p (k w) -> p k w", w=TB_W),
                        in_ap=tfull[:, :],
                        idxs_ap=bsid_sb[:, ofs_b[s] * 8:(ofs_b[s] + K) * 8],
                        num_idxs=K * P, num_idxs_reg=K * P, elem_size=TB_W)
                    udblk = wpool.tile([P, 2], f32, name="udblk", tag="udblk")
                    nc.sync.dma_start(
                        out=udblk[:],
                        in_=tb_loc[l][s * P:(s + 1) * P, HID + 2:HID + 4])
                    t0 = ofs_b[s]
                    # one-hots for all tiles of this slot (kept for 2nd loop)
                    ob = obpool.tile([P, K * P], f32, name="ob", tag="ob")
                    psu = psU.tile([P, 2 * K], f32, name="psu", tag="psu")
                    for k in range(K):
                        onehot(ob[:, k * P:(k + 1) * P],
                               bcid_sb[:, t0 + k:t0 + k + 1])
                        pso = psT.tile([P, P], f32, name="pso", tag="pst")
                        nc.tensor.transpose(out=pso[:],
                                            in_=ob[:, k * P:(k + 1) * P],
                                            identity=ident[:])
                        ot = wpool.tile([P, P], f32, name="ot", tag="sh")
                        nc.vector.tensor_copy(ot[:], pso[:])
                        nc.tensor.matmul(out=psu[:, 2 * k:2 * k + 2],
                                         lhsT=ot[:], rhs=udblk[:],
                                         start=True, stop=True)
                    # e_u for the whole slot
                    tt = wpool.tile([P, 2 * K], f32, name="tt", tag="eu")
                    nc.vector.tensor_tensor(
                        out=tt[:].rearrange("p (k two) -> p k two", two=2),
                        in0=gb[:, 0:K * TB_W].rearrange(
                            "p (k w) -> p k w", w=TB_W)[:, :, HID:HID + 2],
                        in1=psu[:].rearrange("p (k two) -> p k two", two=2),
                        op=AO.add)
                    t2 = wpool.tile([P, 2 * K], f32, name="t2", tag="eu")
                    nc.vector.tensor_tensor(
                        out=t2[:], in0=tt[:],
                        in1=bue_sb[l][:, t0 * 2:(t0 + K) * 2], op=AO.add)
                    t3 = wpool.tile([P, 2 * K], f32, name="t3", tag="eu")
                    nc.vector.tensor_scalar_mul(t3[:], t2[:], 0.2)
                    t4 = wpool.tile([P, 2 * K], f32, name="t4", tag="eu")
                    nc.vector.tensor_tensor(out=t4[:], in0=t2[:], in1=t3[:],
                                            op=AO.max)
                    eu = wpool.tile([P, 2 * K], f32, name="eu", tag="eu")
                    nc.scalar.activation(eu[:], t4[:], AF.Exp)
                    psd = psU.tile([P, 2], f32, name="psd", tag="psd")
                    if _dbg and l == 0 and s == 0:
                        psu_sb = wpool.tile([P, 2 * K], f32, name="psu_sb",
                                            tag="eu")
                        nc.vector.tensor_copy(psu_sb[:], psu[:])
                        us_sb = wpool.tile([P, 2 * K], f32, name="us_sb",
                                           tag="eu")
                        nc.vector.tensor_copy(
                            us_sb[:].rearrange("p (k two) -> p k two", two=2),
                            gb[:, 0:K * TB_W].rearrange(
                                "p (k w) -> p k w", w=TB_W)[:, :, HID:HID + 2])
                        nc.sync.dma_start(out=dbg_eu[:, :], in_=eu[:])
                        nc.sync.dma_start(out=dbg_ud[:, :], in_=psu_sb[:])
                        nc.sync.dma_start(out=dbg_udblk[:, :], in_=udblk[:])
                        nc.sync.dma_start(out=dbg_us[:, :], in_=us_sb[:])
                    ps = psA.tile([P, HID], f32, name="psB_t", tag="psA")
                    for k in range(K):
                        vs = wpool.tile([P, HID], f32, name="vs", tag="vs")
                        for h in range(2):
                            nc.vector.tensor_scalar_mul(
                                vs[:, h * P:(h + 1) * P],
                                gb[:, k * TB_W + h * P:k * TB_W + (h + 1) * P],
                                eu[:, 2 * k + h:2 * k + h + 1])
                        nc.tensor.matmul(out=ps[:, 0:HID],
                                         lhsT=ob[:, k * P:(k + 1) * P],
                                         rhs=vs[:], start=(k == 0),
                                         stop=(k == K - 1))
                        nc.tensor.matmul(out=psd[:],
                                         lhsT=ob[:, k * P:(k + 1) * P],
                                         rhs=eu[:, 2 * k:2 * k + 2],
                                         start=(k == 0), stop=(k == K - 1))
                        if _dbg and l == 0 and s == 0 and k == 0:
                            nc.sync.dma_start(out=dbg_vs0[:, :], in_=vs[:])
                            nc.sync.dma_start(out=dbg_ob0[:, :],
                                              in_=ob[:, 0:P])
                    if _dbg and l == 0 and s == 0:
                        ps_sb = wpool.tile([P, HID], f32, name="ps_sb",
                                           tag="fsb")
                        nc.vector.tensor_copy(ps_sb[:], ps[:])
                        nc.sync.dma_start(out=dbg_ps[:, 0:HID], in_=ps_sb[:])
                    den = wpool.tile([P, 2], f32, name="den", tag="den")
                    nc.vector.tensor_scalar_add(den[:], psd[:], 1e-9)
                    rcp = wpool.tile([P, 2], f32, name="rcp", tag="den")
                    nc.vector.reciprocal(rcp[:], den[:])
                    hmix = wpool.tile([P, HID], f32, name="hmix", tag="vs")
                    for h in range(2):
                        nc.vector.tensor_scalar_mul(
                            hmix[:, h * P:(h + 1) * P],
                            ps[:, h * P:(h + 1) * P], rcp[:, h:h + 1])
                    if residual is None:
                        nc.vector.tensor_copy(
                            dst_res[:, s * HID:(s + 1) * HID], hmix[:])
                    else:
                        nc.vector.tensor_tensor(
                            out=dst_res[:, s * HID:(s + 1) * HID],
                            in0=hmix[:], in1=residual[:, s * HID:(s + 1) * HID],
                            op=AO.add)

            if _lvl >= 4:
                gat_layer(0, h1g, None)

            # gelu (tanh approx) per slot, in place on h1g
            for s in range(QBM if _lvl >= 5 else 0):
                xs = h1g[:, s * HID:(s + 1) * HID]
                x2 = wpool.tile([P, HID], f32, name="gx2", tag="gelu")
                nc.vector.tensor_tensor(out=x2[:], in0=xs, in1=xs, op=AO.mult)
                x3 = wpool.tile([P, HID], f32, name="gx3", tag="gelu")
                nc.vector.tensor_tensor(out=x3[:], in0=x2[:], in1=xs, op=AO.mult)
                zz = wpool.tile([P, HID], f32, name="gzz", tag="gelu")
                nc.vector.tensor_scalar_mul(zz[:], x3[:], 0.044715)
                z4 = wpool.tile([P, HID], f32, name="gz4", tag="gelu")
                nc.vector.tensor_tensor(out=z4[:], in0=zz[:], in1=xs, op=AO.add)
                th = wpool.tile([P, HID], f32, name="gth", tag="gelu")
                nc.scalar.activation(th[:], z4[:], AF.Tanh,
                                     scale=0.7978845608028654)
                uu = wpool.tile([P, HID], f32, name="guu", tag="gelu")
                nc.vector.tensor_scalar(uu[:], th[:], 0.5, 0.5, AO.mult, AO.add)
                nc.vector.tensor_tensor(out=xs, in0=xs, in1=uu[:], op=AO.mult)

            if _lvl >= 5:
                fold(h1g, w_tb_sb[1], 260, tb_loc[1])
                nc.gpsimd.collective_compute(
                    "AllGather", AO.bypass, replica_groups=RG,
                    ins=[tb_loc[1].opt()], outs=[tb_full[1].opt()])

            if _lvl >= 6:
                gat_layer(1, xproc, xlat)

            if _lvl >= 7:
                fold(xproc, w_tc_sb, IN + 1, tc_loc, bias_dram=hl)
                nc.gpsimd.collective_compute(
                    "AllGather", AO.bypass, replica_groups=RG,
                    ins=[tc_loc.opt()], outs=[tc_full.opt()])

            # ---------- phase 5: decoder (stage C) ----------
            ofs_c = np.cumsum([0] + KC)
            for s in range(QBE if _lvl >= 8 else 0):
                K = KC[s]
                if K == 0:
                    continue
                gb = gpool.tile([P, GMAX], f32, name="gbC", tag="gb")
                nc.gpsimd.dma_gather(
                    out_ap=gb[:, 0:K * TC_W].rearrange(
                        "p (k w) -> p k w", w=TC_W),
                    in_ap=tc_full[:, :],
                    idxs_ap=csid_sb[:, ofs_c[s] * 8:(ofs_c[s] + K) * 8],
                    num_idxs=K * P, num_idxs_reg=K * P, elem_size=TC_W)
                t0 = ofs_c[s]
                tt = wpool.tile([P, K], f32, name="ttC", tag="eu")
                nc.vector.tensor_tensor(
                    out=tt[:],
                    in0=gb[:, 0:K * TC_W].rearrange(
                        "p (k w) -> p k w", w=TC_W)[:, :, IN],
                    in1=cuce_sb[:, t0:t0 + K], op=AO.add)
                t3 = wpool.tile([P, K], f32, name="t3C", tag="eu")
                nc.vector.tensor_scalar_mul(t3[:], tt[:], 0.2)
                t4 = wpool.tile([P, K], f32, name="t4C", tag="eu")
                nc.vector.tensor_tensor(out=t4[:], in0=tt[:], in1=t3[:],
                                        op=AO.max)
                eu = wpool.tile([P, K], f32, name="euC", tag="eu")
                nc.scalar.activation(eu[:], t4[:], AF.Exp)
                ps = psA.tile([P, IN], f32, name="psC_t", tag="psA")
                psd = psU.tile([P, 1], f32, name="psdC", tag="psd")
                for k in range(K):
                    O = wpool.tile([P, P], f32, name="O_C", tag="oh")
                    onehot(O[:], ccid_sb[:, t0 + k:t0 + k + 1])
                    vs = wpool.tile([P, IN], f32, name="vsC", tag="vs")
                    nc.vector.tensor_scalar_mul(
                        vs[:], gb[:, k * TC_W:k * TC_W + IN],
                        eu[:, k:k + 1])
                    nc.tensor.matmul(out=ps[:, 0:IN],
                                     lhsT=O[:], rhs=vs[:],
                                     start=(k == 0), stop=(k == K - 1))
                    nc.tensor.matmul(out=psd[:],
                                     lhsT=O[:], rhs=eu[:, k:k + 1],
                                     start=(k == 0), stop=(k == K - 1))
                den = wpool.tile([P, 1], f32, name="denC", tag="den")
                nc.vector.tensor_scalar_add(den[:], psd[:], 1e-9)
                rcp = wpool.tile([P, 1], f32, name="rcpC", tag="den")
                nc.vector.reciprocal(rcp[:], den[:])
                osb = wpool.tile([P, IN], f32, name="osb", tag="vs")
                nc.vector.tensor_scalar_mul(osb[:], ps[:, 0:IN], rcp[:, 0:1])
                nc.sync.dma_start(out=out_t[s * P:(s + 1) * P, :], in_=osb[:])

            if _dbg:
                nc.sync.dma_start(out=dbg_xlat[:, :], in_=xlat[:])
                nc.sync.dma_start(out=dbg_h1g[:, :], in_=h1g[:])
                nc.sync.dma_start(out=dbg_xproc[:, :], in_=xproc[:])
                nc.sync.dma_start(out=dbg_tb1[:, :], in_=tb_full[0][:])

    nc.compile()
    return nc


# ---------------- entry point ----------------

def _make_in_maps(pk):
    in_maps = []
    for c in range(8):
        pc = pk.cores[c]
        m = {
            "xinT": pc.xinT,
            "w_ta": pk.w_ta,
            "w_tb0": pk.w_tb[0], "w_tb1": pk.w_tb[1],
            "w_tc": pk.w_tc,
            "hl": np.ascontiguousarray(pc.hl),
            "a_slo": pc.A.sidx_lo, "a_shi": pc.A.sidx_hi,
            "a_cidx": pc.A.cidx, "a_alpha": pc.A.alpha,
            "b_sidx": pc.B.sidx, "b_cidx": pc.B.cidx,
            "b_ue0": pc.B.streams[0], "b_ue1": pc.B.streams[1],
            "c_sidx": pc.C.sidx, "c_cidx": pc.C.cidx,
            "c_uce": pc.C.streams[0],
        }
        in_maps.append({k: np.ascontiguousarray(v) for k, v in m.items()})
    return in_maps


def kernel(**inputs):
    from concourse.bass_utils import run_bass_kernel_spmd

    pk = _host_prep(inputs)
    nc = _build(pk)
    in_maps = _make_in_maps(pk)
    res = run_bass_kernel_spmd(nc, in_maps, core_ids=list(range(8)))

    x = np.asarray(inputs["x"], np.float32)
    out = np.zeros((BS, ERA, IN), np.float32)
    for g in range(BS):
        quarter = [res.results[g * 4 + r]["out"] for r in range(4)]
        full = np.concatenate(quarter, 0)[:ERA]
        out[g] = full + x[g, :, :IN]
    return out



# revision 62
# speedup vs baseline: 1.9955x; 1.9955x over previous
"""Trainium2 Bass kernel for nn_MixedTransformer (GNN encode-process-decode).

Distribution: 8 cores = 2 batch groups x 4 dst-range quarters.
Per core: dense val-table matmul over the *used* grid rows only, edge gathers
via dma_gather (bf16 tables), segment-softmax message passing via one-hot
matmuls into PSUM, GAT processor with bf16 table all-gathers inside each
4-core group, decoder back to the grid.

Self-contained: hardcodes all shapes; host does edge sorting/packing and the
encoder's softmax weights (all inputs to that stage are host-visible).
"""
import sys

try:
    import concourse  # noqa: F401
except ImportError:
    sys.path.insert(0, "/opt/trn_rl_repo")

import numpy as np

# ---------------- problem constants ----------------
P = 128
BS = 2
ERA, HMESH = 35718, 10242
IN, AUX, POS = 96, 2, 4
HID, HEADS, DH = 256, 2, 128
E_E2H, E_H2H, E_H2E = 107154, 61440, 107154

ERA_PAD, NBE = 35840, 280          # padded grid rows / dst blocks
MH_PAD, NBM = 10752, 84            # padded mesh rows / dst blocks
QBM, QBE = 21, 70                  # dst blocks per quarter (mesh / grid)
QROWS = QBM * P                    # mesh rows per quarter

TA_W = 256                         # T_A row: val(256) bf16
TB_W = 256                         # T_l row: q(256) bf16
TC_W = 128                         # T_C row: val(96) uS(1) pad(31) bf16

GMAX_T = 8                         # dma_gather hard limit: 1024 indices

RG = [[0, 1, 2, 3], [4, 5, 6, 7]]


# ---------------- host-side packing ----------------

def _seg_softmax_host(logits, seg, n):
    """Exact reference segment softmax (f64), returns per-edge alpha."""
    lg = logits.astype(np.float64)
    m = np.full(n, -np.inf)
    np.maximum.at(m, seg, lg)
    e = np.exp(lg - m[seg])
    s = np.zeros(n)
    np.add.at(s, seg, e)
    return (e / (s[seg] + 1e-9)).astype(np.float64)


def _block_partition(src, dst, nblocks, qb):
    """Group edges by 128-row dst block; per program slot s (0..qb-1) compute
    uniform tile counts K (max over the 4 quarters)."""
    blk = dst // P
    order = np.argsort(blk, kind="stable")
    bo = blk[order]
    starts = np.searchsorted(bo, np.arange(nblocks + 1))
    per_block = [order[starts[j]:starts[j + 1]] for j in range(nblocks)]
    K = [max(-(-len(per_block[qb * r + s]) // P) for r in range(4))
         for s in range(qb)]
    # true max edge count per slot across the 4 quarters (descriptor skip)
    N16 = [max(len(per_block[qb * r + s]) for r in range(4))
           for s in range(qb)]
    return per_block, K, N16


def _chunk_slots(K, budget):
    """Greedily group slot indices so each chunk's tile total <= budget."""
    chunks, cur, tot = [], [], 0
    for s, k in enumerate(K):
        if cur and tot + k > budget:
            chunks.append(cur)
            cur, tot = [], 0
        cur.append(s)
        tot += k
    if cur:
        chunks.append(cur)
    return chunks


def _wrap_idx16(idx_flat):
    """Pack int indices for dma_gather: idx j -> [j%16, j//16], tiled to 128
    partitions. idx_flat length must be a multiple of 128."""
    n = len(idx_flat)
    cols = n // 16
    arr = np.zeros((16, cols), np.int16)
    arr[np.arange(n) % 16, np.arange(n) // 16] = idx_flat
    return np.tile(arr, (8, 1))


def _pad_to(arr, n, fill):
    out = np.full(n, fill, arr.dtype)
    out[:len(arr)] = arr
    return out


class _Packed:
    pass


def _bf16():
    import concourse.mybir as mybir
    return np.dtype(mybir.dt.np(mybir.dt.bfloat16))


def _host_prep(inputs):
    f32 = np.float32
    bf16 = _bf16()
    x = np.asarray(inputs["x"], f32)
    e2h = np.asarray(inputs["e2h_idx"]).astype(np.int64)
    h2h = np.asarray(inputs["h2h_idx"]).astype(np.int64)
    h2e = np.asarray(inputs["h2e_idx"]).astype(np.int64)
    e2h_attr = np.asarray(inputs["e2h_attr"], f32)
    h2h_attr = np.asarray(inputs["h2h_attr"], f32)
    h2e_attr = np.asarray(inputs["h2e_attr"], f32)
    era_ll = np.asarray(inputs["era_latlons"], f32)
    h_ll = np.asarray(inputs["h_latlons"], f32)
    fm_ctx = np.asarray(inputs["fm_ctx"], f32)
    fm_Wsrc = np.asarray(inputs["fm_Wsrc"], f32)
    fm_Wctx = np.asarray(inputs["fm_Wctx"], f32)
    fm_Wedge = np.asarray(inputs["fm_Wedge"], f32)
    fm_att = np.asarray(inputs["fm_att"], f32)
    fm_Wval = np.asarray(inputs["fm_Wval"], f32)
    bm_ctx = np.asarray(inputs["bm_ctx"], f32)
    bm_Wsrc = np.asarray(inputs["bm_Wsrc"], f32)
    bm_Wctx = np.asarray(inputs["bm_Wctx"], f32)
    bm_Wedge = np.asarray(inputs["bm_Wedge"], f32)
    bm_att = np.asarray(inputs["bm_att"], f32)
    bm_Wval = np.asarray(inputs["bm_Wval"], f32)
    gat_W = np.asarray(inputs["gat_W"], f32)
    gat_We = np.asarray(inputs["gat_We"], f32)
    gat_asrc = np.asarray(inputs["gat_asrc"], f32)
    gat_adst = np.asarray(inputs["gat_adst"], f32)
    gat_aedge = np.asarray(inputs["gat_aedge"], f32)

    pk = _Packed()

    # ---- encoder (stage A): host computes exact per-edge alpha ----
    sA, dA = e2h[0], e2h[1]
    x_in = [np.concatenate([x[g].reshape(ERA, IN + AUX), era_ll], 1)
            for g in range(BS)]                                   # (35718,102)
    fm_w_att = fm_Wsrc @ fm_att                                   # (102,)
    uC_A = np.concatenate([fm_ctx, h_ll], 1) @ (fm_Wctx @ fm_att)  # (HMESH,)
    uE_A = e2h_attr @ (fm_Wedge @ fm_att)                         # (E,)
    alphas_A = []
    for g in range(BS):
        uS = x_in[g] @ fm_w_att                                   # (ERA,)
        logit = uS[sA] + uC_A[dA] + uE_A
        lrelu = np.where(logit >= 0, logit, 0.2 * logit)
        alphas_A.append(_seg_softmax_host(lrelu, dA, HMESH))

    pbA, KA, NA16 = _block_partition(sA, dA, NBM, QBM)

    # unique used src rows per quarter (shared across both batch groups)
    U = []
    for r in range(4):
        e = np.concatenate([pbA[QBM * r + s] for s in range(QBM)])
        U.append(np.unique(sA[e]))
    NBU = max(-(-len(u) // P) for u in U)

    # ---- processor (stage B) ----
    sB, dB = h2h[0], h2h[1]
    pbB, KB_, NB16_ = _block_partition(sB, dB, NBM, QBM)
    # split each block's edges by source half-of-quarter (chunked AG overlap)
    halfB = (sB % 2688) >= 1408
    pbB0 = [e[~halfB[e]] for e in pbB]
    pbB1 = [e[halfB[e]] for e in pbB]
    KB0 = [max(-(-len(pbB0[QBM * r + s]) // P) for r in range(4))
           for s in range(QBM)]
    KB1 = [max(-(-len(pbB1[QBM * r + s]) // P) for r in range(4))
           for s in range(QBM)]
    NB0 = [max(len(pbB0[QBM * r + s]) for r in range(4)) for s in range(QBM)]
    NB1 = [max(len(pbB1[QBM * r + s]) for r in range(4)) for s in range(QBM)]
    KB = [KB0[s] + KB1[s] for s in range(QBM)]
    pk.KB0, pk.KB1, pk.NB0, pk.NB1 = KB0, KB1, NB0, NB1
    uE_B = [h2h_attr @ np.einsum("fhd,hd->fh", gat_We[l], gat_aedge[l])
            for l in range(2)]                                    # (E,2)
    w_d = [np.einsum("fhd,hd->fh", gat_W[l], gat_adst[l]) for l in range(2)]
    # fold weights: q(256) + ud(2)
    pk.w_tb = [np.concatenate(
        [gat_W[l].reshape(HID, HID), w_d[l]], 1).astype(bf16) for l in range(2)]
    # per-edge uS = q . a_src, computed on device from gathered q.
    # physically replicated so the batched multiply needs no broadcast AP
    KMB_ = max(KB)
    pk.a_bc = [np.tile(gat_asrc[l].reshape(1, HID), (P, KMB_)).astype(bf16)
               for l in range(2)]                                 # (128,KMB*256)

    # ---- decoder (stage C) ----
    sC, dC = h2e[0], h2e[1]
    pbC, KC, NC16 = _block_partition(sC, dC, NBE, QBE)
    bm_w_att = bm_Wsrc @ bm_att                                   # (260,)
    uC_C = np.concatenate([bm_ctx, era_ll], 1) @ (bm_Wctx @ bm_att)  # (ERA,)
    uE_C = h2e_attr @ (bm_Wedge @ bm_att)                         # (E,)
    uCE_C = uC_C[dC] + uE_C

    pk.w_tc = np.concatenate(
        [bm_Wval[:HID], bm_w_att[:HID, None]], 1).astype(bf16)    # (256,97)
    hl_term = h_ll @ np.concatenate(
        [bm_Wval[HID:], bm_w_att[HID:, None]], 1)                 # (HMESH,97)
    hl_pad = np.zeros((MH_PAD, IN + 1), f32)
    hl_pad[:HMESH] = hl_term
    # per-core bias stream, partition-major: hl[p, s*97+c] = hl_pad[r*2688+s*128+p, c]
    pk.hl = []
    for r in range(4):
        q = hl_pad[QROWS * r:QROWS * (r + 1)].reshape(QBM, P, IN + 1)
        pk.hl.append(np.ascontiguousarray(
            q.transpose(1, 0, 2).reshape(P, QBM * (IN + 1))).astype(bf16))

    pk.w_ta = fm_Wval.astype(bf16)                                # (102,256)

    pk.KA, pk.KB, pk.KC = KA, KB, KC
    pk.NA16, pk.NC16 = NA16, NC16
    SKA, SKB, SKC = sum(KA), sum(KB), sum(KC)
    pk.SKA, pk.SKB, pk.SKC = SKA, SKB, SKC
    pk.NBU = NBU
    pk.KMA, pk.KMB, pk.KMC = max(KA), max(KB), max(KC)
    # stage-C slot groups (shared gather per group, <= 8 tiles)
    grpC, cur, tot = [], [], 0
    for s in range(QBE):
        if cur and tot + KC[s] > 8:
            grpC.append(cur)
            cur, tot = [], 0
        cur.append(s)
        tot += KC[s]
    if cur:
        grpC.append(cur)
    pk.grpC = grpC
    last_of_group = set(g[-1] for g in grpC)
    pk.KMAX = max(pk.KMA, pk.KMB, pk.KMC)

    # iota row template replicated for up to KMAX tiles
    iota_t = np.tile(np.arange(P, dtype=f32), pk.KMAX)
    pk.iota_t = np.tile(iota_t[None, :], (P, 1)).astype(bf16)     # (128,KMAX*128)

    def pack_quarter_A(r, g):
        sidx, cidx, alph = [], [], []
        remap = np.full(ERA, 0, np.int64)
        remap[U[r]] = np.arange(len(U[r]))
        for s in range(QBM):
            j = QBM * r + s
            e = pbA[j]
            n = KA[s] * P
            si = np.full(n, -1, np.int16)
            si[:NA16[s]] = 0
            si[:len(e)] = remap[sA[e]].astype(np.int16)
            sidx.append(si)
            cidx.append(_pad_to((dA[e] - j * P).astype(f32), n, -1.0))
            alph.append(_pad_to(alphas_A[g][e].astype(f32), n, 0.0))
        out = _Packed()
        out.sidx = _wrap_idx16(np.concatenate(sidx))
        out.cidx = np.concatenate(cidx).reshape(SKA, P).T.copy()
        out.alpha = np.concatenate(alph).reshape(SKA, P).T.copy()
        return out

    def pack_quarter_BC(r, per_block, K, qb, src, dst, SK, streams, cbc,
                        nmax, tail_neg=None):
        """streams: list of per-edge arrays (E,) or (E,m) -> packed (128, SK*m)."""
        sidx, cidx, st_out = [], [], [[] for _ in streams]
        for s in range(qb):
            j = qb * r + s
            e = per_block[j]
            n = K[s] * P
            if tail_neg is None or s in tail_neg:
                si = np.full(n, -1, np.int16)
                si[:nmax[s]] = 0
            else:
                si = np.zeros(n, np.int16)
            si[:len(e)] = src[e].astype(np.int16)
            sidx.append(si)
            cidx.append(_pad_to((dst[e] - j * P).astype(f32), n, -1.0))
            for q, arr in enumerate(streams):
                a = arr[e]
                if a.ndim == 1:
                    a = a[:, None]
                m = a.shape[1]
                buf = np.zeros((n, m), f32)
                buf[:len(e)] = a
                st_out[q].append(buf)
        out = _Packed()
        out.sidx = _wrap_idx16(np.concatenate(sidx))
        cf = np.concatenate(cidx).reshape(SK, P).T
        out.cidx = cf.copy()
        if cbc:
            # broadcast cidx: [p, t*128+c] = cidx of edge (t, c), same all p
            out.cbc = np.ascontiguousarray(
                np.tile(cf.T.reshape(1, SK * P), (P, 1))).astype(bf16)
        out.streams = []
        for q, parts in enumerate(st_out):
            a = np.concatenate(parts, 0)                          # (SK*P, m)
            m = a.shape[1]
            out.streams.append(
                a.reshape(SK, P, m).transpose(1, 0, 2).reshape(P, SK * m).copy())
        return out

    def pack_B(r):
        sidx0, sidx1, cidx, ues = [], [], [], [[], []]
        for s in range(QBM):
            j = QBM * r + s
            for half, (pb, Kh, Nh, sil, rows, off) in enumerate(
                    [(pbB0, KB0, NB0, sidx0, 1408, 0),
                     (pbB1, KB1, NB1, sidx1, 1280, 1408)]):
                e = pb[j]
                n = Kh[s] * P
                if n == 0:
                    continue
                m = sB[e]
                rem = (m // 2688) * rows + (m % 2688) - off
                si = np.full(n, -1, np.int16)
                si[:Nh[s]] = 0
                si[:len(e)] = rem.astype(np.int16)
                sil.append(si)
                cidx.append(_pad_to((dB[e] - j * P).astype(f32), n, -1.0))
                for l in range(2):
                    buf = np.zeros((n, 2), f32)
                    buf[:len(e)] = uE_B[l][e]
                    ues[l].append(buf)
        out = _Packed()
        out.sidx0 = _wrap_idx16(np.concatenate(sidx0))
        out.sidx1 = _wrap_idx16(np.concatenate(sidx1))
        cf = np.concatenate(cidx).reshape(SKB, P).T
        out.cidx = cf.copy()
        out.cbc = np.ascontiguousarray(
            np.tile(cf.T.reshape(1, SKB * P), (P, 1))).astype(bf16)
        out.streams = []
        for l in range(2):
            a = np.concatenate(ues[l], 0)
            out.streams.append(
                a.reshape(SKB, P, 2).transpose(1, 0, 2).reshape(
                    P, SKB * 2).copy())
        return out

    pk.cores = []
    for c in range(8):
        g, r = c // 4, c % 4
        pc = _Packed()
        pc.A = pack_quarter_A(r, g)
        pc.B = pack_B(r)
        pc.C = pack_quarter_BC(r, pbC, KC, QBE, sC, dC, SKC,
                               [uCE_C], cbc=False, nmax=NC16,
                               tail_neg=last_of_group)
        pc.hl = pk.hl[r]
        # compact transposed encoder input, feature-major: (102, NBU*128)
        xt = np.zeros((IN + AUX + POS, NBU * P), f32)
        xt[:, :len(U[r])] = x_in[g][U[r]].T
        pc.xinT = xt.astype(bf16)
        pk.cores.append(pc)
    return pk


# ---------------- device program ----------------

def _build(pk):
    import concourse.mybir as mybir
    import concourse.tile as tile
    from concourse import bacc
    from concourse.masks import make_identity

    f32 = mybir.dt.float32
    bf16 = mybir.dt.bfloat16
    i16 = mybir.dt.int16
    AO = mybir.AluOpType
    AF = mybir.ActivationFunctionType

    nc = bacc.Bacc("TRN2", target_bir_lowering=False, debug=False,
                   num_devices=8)

    SKA, SKB, SKC = pk.SKA, pk.SKB, pk.SKC
    KA, KB, KC = pk.KA, pk.KB, pk.KC
    NA16, NC16 = pk.NA16, pk.NC16
    KB0, KB1, NB0, NB1 = pk.KB0, pk.KB1, pk.NB0, pk.NB1
    NBU, KMAX = pk.NBU, pk.KMAX
    KMA, KMB, KMC = pk.KMA, pk.KMB, pk.KMC
    IN_F = IN + AUX + POS
    ein = {}

    def xin(name, shape, dt=bf16):
        ein[name] = nc.dram_tensor(name, shape, dt, kind="ExternalInput")
        return ein[name]

    xinT = xin("xinT", [IN_F, NBU * P])
    w_ta = xin("w_ta", [IN_F, TA_W])
    w_tb0 = xin("w_tb0", [HID, HID + 2])
    w_tb1 = xin("w_tb1", [HID, HID + 2])
    a_bc0 = xin("a_bc0", [P, KMB * HID])
    a_bc1 = xin("a_bc1", [P, KMB * HID])
    w_tc = xin("w_tc", [HID, IN + 1])
    hl = xin("hl", [P, QBM * (IN + 1)])
    iota_in = xin("iota_t", [P, KMAX * P])
    a_sidx = xin("a_sidx", [P, SKA * 8], i16)
    a_cidx = xin("a_cidx", [P, SKA], f32)
    a_alpha = xin("a_alpha", [P, SKA], f32)
    SKB0, SKB1 = sum(pk.KB0), sum(pk.KB1)
    b_sidx0 = xin("b_sidx0", [P, SKB0 * 8], i16)
    b_sidx1 = xin("b_sidx1", [P, SKB1 * 8], i16)
    b_cidx = xin("b_cidx", [P, SKB], f32)
    b_cbc = xin("b_cbc", [P, SKB * P])
    b_ue0 = xin("b_ue0", [P, SKB * 2], f32)
    b_ue1 = xin("b_ue1", [P, SKB * 2], f32)
    c_sidx = xin("c_sidx", [P, SKC * 8], i16)
    c_cidx = xin("c_cidx", [P, SKC], f32)
    c_uce = xin("c_uce", [P, SKC], f32)
    out_t = nc.dram_tensor("out", [P, QBE * IN], bf16, kind="ExternalOutput")
    import os
    _lvl = int(os.environ.get("KERNEL_PHASES", "8"))
    _sub = int(os.environ.get("KERNEL_SUB", "4"))

    ofs_a = np.cumsum([0] + KA)
    ofs_b = np.cumsum([0] + KB)
    ofs_c = np.cumsum([0] + KC)

    with tile.TileContext(nc) as tc:
        with tc.tile_pool(name="const", bufs=1) as cpool, \
             tc.tile_pool(name="stream", bufs=1) as spool, \
             tc.tile_pool(name="res", bufs=1) as rpool, \
             tc.tile_pool(name="gat", bufs=3) as gpool, \
             tc.tile_pool(name="lx", bufs=2) as lxpool, \
             tc.tile_pool(name="work", bufs=3) as wpool, \
             tc.tile_pool(name="ob", bufs=3) as obpool, \
             tc.tile_pool(name="psA", bufs=3, space="PSUM") as psA, \
             tc.tile_pool(name="psU", bufs=2, space="PSUM") as psU, \
             tc.tile_pool(name="psT", bufs=1, space="PSUM") as psT, \
             tc.tile_pool(name="psF", bufs=2, space="PSUM") as psF, \
             tc.tile_pool(name="dram", bufs=1, space="DRAM") as dpool:

            # ---------- constants / streams ----------
            ident = cpool.tile([P, P], bf16, name="ident")
            make_identity(nc, ident[:])

            def load(name, src, shape, dt=bf16):
                t = spool.tile(shape, dt, name=name)
                nc.sync.dma_start(out=t[:], in_=src[tuple(slice(0, s) for s in shape)])
                return t

            w_ta_sb = load("w_ta_sb", w_ta, [IN_F, TA_W])

            def load_half(name, src, h, cols):
                t = spool.tile([P, cols], bf16, name=name)
                nc.sync.dma_start(out=t[:], in_=src[h * P:(h + 1) * P, 0:cols])
                return t[:]

            w_tb_sb = [[load_half(f"w_tb{l}_{h}", [w_tb0, w_tb1][l], h, HID + 2)
                        for h in range(2)] for l in range(2)]
            a_bc_sb = [load(f"a_bc{l}_sb", [a_bc0, a_bc1][l], [P, KMB * HID])
                       for l in range(2)]
            w_tc_sb = [load_half(f"w_tc_{h}", w_tc, h, IN + 1)
                       for h in range(2)]
            hl_sb = load("hl_sb", hl, [P, QBM * (IN + 1)])
            iota_t = load("iota_sb", iota_in, [P, KMAX * P])
            asid_sb = load("asid_sb", a_sidx, [P, SKA * 8], i16)
            acid_sb = load("acid_sb", a_cidx, [P, SKA], f32)
            aal_sb = load("aal_sb", a_alpha, [P, SKA], f32)
            bsid0_sb = load("bsid0_sb", b_sidx0, [P, SKB0 * 8], i16)
            bsid1_sb = load("bsid1_sb", b_sidx1, [P, SKB1 * 8], i16)
            bcid_sb = load("bcid_sb", b_cidx, [P, SKB], f32)
            bcbc_sb = load("bcbc_sb", b_cbc, [P, SKB * P])
            bue_sb = [load("bue0_sb", b_ue0, [P, SKB * 2], f32),
                      load("bue1_sb", b_ue1, [P, SKB * 2], f32)]
            csid_sb = load("csid_sb", c_sidx, [P, SKC * 8], i16)
            ccid_sb = load("ccid_sb", c_cidx, [P, SKC], f32)
            cuce_sb = load("cuce_sb", c_uce, [P, SKC], f32)

            # partition-index column (value = partition id), bf16
            piota_i = cpool.tile([P, 1], mybir.dt.int32, name="piota_i")
            nc.gpsimd.iota(piota_i[:], pattern=[[1, 1]], base=0,
                           channel_multiplier=1)
            piota = cpool.tile([P, 1], bf16, name="piota")
            nc.vector.tensor_copy(piota[:], piota_i[:])

            # persistent gather buffers (manual round-robin, zeroed once
            # so descriptor-skipped pad rows stay finite)
            gbufs = [rpool.tile([P, KMA * TA_W], bf16, name=f"gbuf{i}")
                     for i in range(5)]
            for t in gbufs:
                nc.vector.memset(t[:], 0.0)
            gb_i = [0]

            def next_gb():
                t = gbufs[gb_i[0] % 5]
                gb_i[0] += 1
                return t

            # ---------- resident quarter features (bf16) ----------
            xlat = rpool.tile([P, QBM * HID], bf16, name="xlat")
            h1g = rpool.tile([P, QBM * HID], bf16, name="h1g")
            xproc = rpool.tile([P, QBM * HID], bf16, name="xproc")
            ud_res = [rpool.tile([P, QBM * 2], bf16, name=f"ud{l}")
                      for l in range(2)]
            osb_res = rpool.tile([P, QBE * IN], bf16, name="osb")

            # ---------- DRAM tables (bf16) ----------
            ta_dram = dpool.tile([NBU * P, TA_W], bf16, name="ta_dram")
            HR0, HR1 = 11 * P, 10 * P
            tb_locA = [dpool.tile([HR0, TB_W], bf16, name=f"tb_locA{l}")
                       for l in range(2)]
            tb_locB = [dpool.tile([HR1, TB_W], bf16, name=f"tb_locB{l}")
                       for l in range(2)]
            tb_halfA = [dpool.tile([4 * HR0, TB_W], bf16, name=f"tb_hA{l}")
                        for l in range(2)]
            tb_halfB = [dpool.tile([4 * HR1, TB_W], bf16, name=f"tb_hB{l}")
                        for l in range(2)]
            tc_loc = dpool.tile([QROWS, TC_W], bf16, name="tc_loc")
            tc_full = dpool.tile([MH_PAD, TC_W], bf16, name="tc_full")

            # ---------- phase 1: dense T_A (compact rows) ----------
            CW = 16                               # blocks per input chunk
            for c0 in range(0, NBU if _lvl >= 1 else 0, CW):
                cw = min(CW, NBU - c0)
                lx = lxpool.tile([IN_F, CW * P], bf16, name="lx", tag="lx")
                nc.sync.dma_start(out=lx[:, 0:cw * P],
                                  in_=xinT[:, c0 * P:(c0 + cw) * P])
                for b0 in range(0, cw, 8):
                    bw = min(8, cw - b0)
                    ta_sb = wpool.tile([P, 8 * TA_W], bf16, name="ta_sb",
                                       tag="ta_sb")
                    for k in range(bw):
                        pst = psT.tile([P, TA_W], f32, name="ps_ta", tag="pst")
                        nc.tensor.matmul(
                            out=pst[:],
                            lhsT=lx[:, (b0 + k) * P:(b0 + k + 1) * P],
                            rhs=w_ta_sb[:], start=True, stop=True)
                        nc.scalar.activation(
                            ta_sb[:, k * TA_W:(k + 1) * TA_W], pst[:], AF.Copy)
                    base = (c0 + b0) * P
                    nc.sync.dma_start(
                        out=ta_dram[base:base + bw * P, :].rearrange(
                            "(b p) w -> p b w", p=P),
                        in_=ta_sb[:, 0:bw * TA_W].rearrange(
                            "p (b w) -> p b w", w=TA_W))

            # ---------- helper: one-hot for tile k of a slot ----
            def onehot(dst_ap, cidx_col):
                # dst[p, j] = (cidx[p] == j)
                nc.vector.tensor_tensor(
                    out=dst_ap, in0=cidx_col.to_broadcast([P, P]),
                    in1=iota_t[:, 0:P], op=AO.is_equal)

            # ---------- phase 2: stage A (encoder edges) ----------
            for s in range(QBM if _lvl >= 2 else 0):
                K = KA[s]
                if K == 0:
                    continue
                t0 = ofs_a[s]
                gb = next_gb()
                for c0 in range(0, K, GMAX_T):
                    cw = min(GMAX_T, K - c0)
                    nfull = cw * P
                    nreg = min(max(NA16[s] - c0 * P, 0), nfull)
                    nc.gpsimd.dma_gather(
                        out_ap=gb[:, c0 * TA_W:(c0 + cw) * TA_W].rearrange(
                            "p (k w) -> p k w", w=TA_W),
                        in_ap=ta_dram[:, :],
                        idxs_ap=asid_sb[:, (t0 + c0) * 8:(t0 + c0 + cw) * 8],
                        num_idxs=nfull, num_idxs_reg=nreg,
                        elem_size=TA_W)
                if True:
                    if _sub < 2:
                        continue
                    S = obpool.tile([P, KMA * P], bf16, name="S_A", tag="ob")
                    S3 = S[:, 0:K * P].rearrange("p (k c) -> p k c", c=P)
                    nc.vector.tensor_tensor(
                        out=S3,
                        in0=acid_sb[:, t0:t0 + K].rearrange(
                            "p (k one) -> p k one", one=1).to_broadcast(
                            [P, K, P]),
                        in1=iota_t[:, 0:K * P].rearrange(
                            "p (k c) -> p k c", c=P),
                        op=AO.is_equal)
                    nc.vector.tensor_tensor(
                        out=S3, in0=S3,
                        in1=aal_sb[:, t0:t0 + K].rearrange(
                            "p (k one) -> p k one", one=1).to_broadcast(
                            [P, K, P]),
                        op=AO.mult)
                    if _sub < 3:
                        continue
                    ps = psA.tile([P, HID], f32, name="psA_t", tag="psA")
                    for k in range(K):
                        nc.tensor.matmul(
                            out=ps[:], lhsT=S[:, k * P:(k + 1) * P],
                            rhs=gb[:, k * TA_W:k * TA_W + HID],
                            start=(k == 0), stop=(k == K - 1))
                    if _sub < 4:
                        continue
                    nc.scalar.activation(
                        xlat[:, s * HID:(s + 1) * HID], ps[:], AF.Copy)

            # ---------- helper: fold resident -> table ----------
            def fold_slot(src, w_sb, wcols, s, dst_dram=None, ud_dst=None,
                          bias_sb=None, dst_half=None):
                if True:
                    pst = psT.tile([P, HID], bf16, name="ps_tr", tag="pst")
                    for h in range(2):
                        nc.tensor.transpose(
                            out=pst[:, h * P:(h + 1) * P],
                            in_=src[:, s * HID + h * P:s * HID + (h + 1) * P],
                            identity=ident[:])
                    xt = wpool.tile([P, HID], bf16, name="xt", tag="xt")
                    nc.vector.tensor_copy(xt[:], pst[:])
                    psf = psF.tile([P, wcols], f32, name="ps_f", tag="psf")
                    for h in range(2):
                        nc.tensor.matmul(out=psf[:],
                                         lhsT=xt[:, h * P:(h + 1) * P],
                                         rhs=w_sb[h][:, 0:wcols],
                                         start=(h == 0), stop=(h == 1))
                    fsb = wpool.tile([P, TB_W], bf16, name="fsb", tag="fsb")
                    if bias_sb is not None:
                        nc.vector.tensor_tensor(
                            out=fsb[:, 0:wcols], in0=psf[:, 0:wcols],
                            in1=bias_sb[:, s * wcols:(s + 1) * wcols],
                            op=AO.add)
                        nc.sync.dma_start(
                            out=dst_dram[s * P:(s + 1) * P, 0:wcols],
                            in_=fsb[:, 0:wcols])
                    else:
                        nc.scalar.activation(fsb[:, 0:HID], psf[:, 0:HID],
                                             AF.Copy)
                        da, db_ = dst_half
                        dap = (da[s * P:(s + 1) * P, 0:HID] if s < 11 else
                               db_[(s - 11) * P:(s - 10) * P, 0:HID])
                        nc.sync.dma_start(out=dap, in_=fsb[:, 0:HID])
                        nc.scalar.activation(
                            ud_dst[:, 2 * s:2 * s + 2], psf[:, HID:HID + 2],
                            AF.Copy)


            def ag_half(loc, full):
                nc.gpsimd.collective_compute(
                    "AllGather", AO.bypass, replica_groups=RG,
                    ins=[loc.opt()], outs=[full.opt()])

            if _lvl >= 3:
                fold(xlat, w_tb_sb[0], HID + 2, None, ud_dst=ud_res[0],
                     dst_half=(tb_locA[0], tb_locB[0]),
                     after10=lambda: ag_half(tb_locA[0], tb_halfA[0]))
                ag_half(tb_locB[0], tb_halfB[0])

            # ---------- phase 3/4: GAT layers ----------
            ofs_b0 = np.cumsum([0] + KB0)
            ofs_b1 = np.cumsum([0] + KB1)

            def gat_layer(l, dst_res, residual, after_slot=None):
                for s in range(QBM):
                    K = KB[s]
                    if K == 0:
                        if after_slot is not None:
                            after_slot(s)
                        continue
                    t0 = ofs_b[s]
                    gb = next_gb()
                    for tfull, Kh, Nh, ofs_h, sid in (
                            (tb_halfA[l], KB0, NB0, ofs_b0, bsid0_sb),
                            (tb_halfB[l], KB1, NB1, ofs_b1, bsid1_sb)):
                        if Kh[s] == 0:
                            continue
                        o0 = 0 if tfull is tb_halfA[l] else KB0[s]
                        h0 = ofs_h[s]
                        nc.gpsimd.dma_gather(
                            out_ap=gb[:, o0 * TB_W:
                                      (o0 + Kh[s]) * TB_W].rearrange(
                                "p (k w) -> p k w", w=TB_W),
                            in_ap=tfull[:, :],
                            idxs_ap=sid[:, h0 * 8:(h0 + Kh[s]) * 8],
                            num_idxs=Kh[s] * P, num_idxs_reg=Nh[s],
                            elem_size=TB_W)
                    if True:
                        gsl = gb[:, 0:K * TB_W]
                        O = obpool.tile([P, KMB * P], bf16, name="O_B",
                                        tag="ob")
                        nc.vector.tensor_tensor(
                            out=O[:, 0:K * P].rearrange(
                                "p (k c) -> p k c", c=P),
                            in0=bcid_sb[:, t0:t0 + K].rearrange(
                                "p (k one) -> p k one", one=1).to_broadcast(
                                [P, K, P]),
                            in1=iota_t[:, 0:K * P].rearrange(
                                "p (k c) -> p k c", c=P),
                            op=AO.is_equal)
                        OT = wpool.tile([P, KMB * P], bf16, name="OT",
                                        tag="ot")
                        nc.vector.tensor_tensor(
                            out=OT[:, 0:K * P],
                            in0=piota[:].to_broadcast([P, K * P]),
                            in1=bcbc_sb[:, t0 * P:(t0 + K) * P],
                            op=AO.is_equal)
                        # us[p, 2k+h] = sum_d q[p,k,h,d] * a_src[h,d]
                        tmp = wpool.tile([P, KMB * TB_W], bf16, name="tmp",
                                         tag="tmp")
                        nc.vector.tensor_tensor(
                            out=tmp[:, 0:K * TB_W],
                            in0=gsl,
                            in1=a_bc_sb[l][:, 0:K * TB_W], op=AO.mult)
                        us = wpool.tile([P, 2 * KMAX], f32, name="us",
                                        tag="us")
                        nc.vector.tensor_reduce(
                            out=us[:, 0:2 * K],
                            in_=tmp[:, 0:K * TB_W].rearrange(
                                "p (k h d) -> p k h d", h=2, d=DH),
                            axis=mybir.AxisListType.X, op=AO.add)
                        psu = psU.tile([P, 2 * KMAX], f32, name="psu",
                                       tag="psu")
                        for k in range(K):
                            nc.tensor.matmul(
                                out=psu[:, 2 * k:2 * k + 2],
                                lhsT=OT[:, k * P:(k + 1) * P],
                                rhs=ud_res[l][:, 2 * s:2 * s + 2],
                                start=True, stop=True)
                        t2 = wpool.tile([P, 2 * KMAX], f32, name="t2",
                                        tag="eu")
                        nc.vector.tensor_tensor(
                            out=t2[:, 0:2 * K], in0=us[:, 0:2 * K],
                            in1=psu[:, 0:2 * K], op=AO.add)
                        t3 = wpool.tile([P, 2 * KMAX], f32, name="t3",
                                        tag="eu")
                        nc.vector.tensor_tensor(
                            out=t3[:, 0:2 * K], in0=t2[:, 0:2 * K],
                            in1=bue_sb[l][:, t0 * 2:(t0 + K) * 2], op=AO.add)
                        t4 = wpool.tile([P, 2 * KMAX], f32, name="t4",
                                        tag="eu")
                        nc.vector.scalar_tensor_tensor(
                            out=t4[:, 0:2 * K], in0=t3[:, 0:2 * K],
                            scalar=0.2, in1=t3[:, 0:2 * K],
                            op0=AO.mult, op1=AO.max)
                        eu = wpool.tile([P, 2 * KMAX], f32, name="eu",
                                        tag="eu")
                        nc.scalar.activation(eu[:, 0:2 * K], t4[:, 0:2 * K],
                                             AF.Exp)
                        # vs[p, k*258 + h*128+d] = q * eu ; cols 256:258 = eu
                        vs = wpool.tile([P, KMB * (TB_W + 2)], bf16,
                                        name="vs", tag="vs")
                        vsv = vs[:, 0:K * (TB_W + 2)].rearrange(
                            "p (k w) -> p k w", w=TB_W + 2)
                        nc.vector.tensor_tensor(
                            out=vsv[:, :, 0:TB_W].rearrange(
                                "p k (h d) -> p k h d", h=2),
                            in0=gsl.rearrange("p (k h d) -> p k h d",
                                              h=2, d=DH),
                            in1=eu[:, 0:2 * K].rearrange(
                                "p (k h one) -> p k h one",
                                h=2, one=1).to_broadcast([P, K, 2, DH]),
                            op=AO.mult)
                        nc.scalar.activation(
                            vsv[:, :, TB_W:TB_W + 2],
                            eu[:, 0:2 * K].rearrange("p (k h) -> p k h", h=2),
                            AF.Copy)
                        ps = psA.tile([P, TB_W + 2], f32, name="psB_t",
                                      tag="psA")
                        for k in range(K):
                            nc.tensor.matmul(
                                out=ps[:],
                                lhsT=O[:, k * P:(k + 1) * P],
                                rhs=vs[:, k * (TB_W + 2):(k + 1) * (TB_W + 2)],
                                start=(k == 0), stop=(k == K - 1))
                        den = wpool.tile([P, 2], f32, name="den", tag="den")
                        nc.vector.tensor_scalar_add(den[:], ps[:, TB_W:TB_W + 2],
                                                    1e-9)
                        rcp = wpool.tile([P, 2], f32, name="rcp", tag="den")
                        nc.vector.reciprocal(rcp[:], den[:])
                        for h in range(2):
                            od = dst_res[:, s * HID + h * P:s * HID + (h + 1) * P]
                            if residual is None:
                                nc.scalar.activation(
                                    od, ps[:, h * P:(h + 1) * P], AF.Copy,
                                    scale=rcp[:, h:h + 1])
                            else:
                                nc.vector.scalar_tensor_tensor(
                                    out=od, in0=ps[:, h * P:(h + 1) * P],
                                    scalar=rcp[:, h:h + 1],
                                    in1=residual[:, s * HID + h * P:
                                                 s * HID + (h + 1) * P],
                                    op0=AO.mult, op1=AO.add)
                    if after_slot is not None:
                        after_slot(s)

            # layer-1 prep emitted in halves inside gat0: gelu+fold1+AG
            def after_gat0(s):
                if _lvl < 5 or (s != 10 and s != QBM - 1):
                    return
                lo = 0 if s == 10 else 11
                for s2 in range(lo, s + 1):
                    xs = h1g[:, s2 * HID:(s2 + 1) * HID]
                    nc.scalar.activation(xs, xs, AF.Gelu_apprx_tanh)
                for s2 in range(lo, s + 1):
                    fold_slot(h1g, w_tb_sb[1], HID + 2, s2,
                              ud_dst=ud_res[1],
                              dst_half=(tb_locA[1], tb_locB[1]))
                if s == 10:
                    ag_half(tb_locA[1], tb_halfA[1])
                else:
                    ag_half(tb_locB[1], tb_halfB[1])

            if _lvl >= 4:
                gat_layer(0, h1g, None, after_slot=after_gat0)

            def after_gat1(s):
                if _lvl >= 7:
                    fold_slot(xproc, w_tc_sb, IN + 1, s, dst_dram=tc_loc,
                              bias_sb=hl_sb)

            if _lvl >= 6:
                gat_layer(1, xproc, xlat, after_slot=after_gat1)

            if _lvl >= 7:
                nc.gpsimd.collective_compute(
                    "AllGather", AO.bypass, replica_groups=RG,
                    ins=[tc_loc.opt()], outs=[tc_full.opt()])

            # ---------- phase 5: decoder (stage C) ----------
            grpC = pk.grpC
            flushC = 0
            for grp in (grpC if _lvl >= 8 else []):
                t0 = ofs_c[grp[0]]
                KT = ofs_c[grp[-1] + 1] - t0
                nreg = (ofs_c[grp[-1]] - t0) * P + NC16[grp[-1]]
                gb = next_gb()
                nc.gpsimd.dma_gather(
                    out_ap=gb[:, 0:KT * TC_W].rearrange(
                        "p (k w) -> p k w", w=TC_W),
                    in_ap=tc_full[:, :],
                    idxs_ap=csid_sb[:, t0 * 8:(t0 + KT) * 8],
                    num_idxs=KT * P, num_idxs_reg=nreg,
                    elem_size=TC_W)
                gsl = gb[:, 0:KT * TC_W]
                tt = wpool.tile([P, GMAX_T], f32, name="ttC", tag="eu")
                nc.vector.tensor_tensor(
                    out=tt[:, 0:KT],
                    in0=gsl.rearrange("p (k w) -> p k w",
                                      w=TC_W)[:, :, IN],
                    in1=cuce_sb[:, t0:t0 + KT], op=AO.add)
                t4 = wpool.tile([P, GMAX_T], f32, name="t4C", tag="eu")
                nc.vector.scalar_tensor_tensor(
                    out=t4[:, 0:KT], in0=tt[:, 0:KT], scalar=0.2,
                    in1=tt[:, 0:KT], op0=AO.mult, op1=AO.max)
                eu = wpool.tile([P, GMAX_T], f32, name="euC", tag="eu")
                nc.scalar.activation(eu[:, 0:KT], t4[:, 0:KT], AF.Exp)
                O = obpool.tile([P, GMAX_T * P], bf16, name="O_C", tag="ob")
                vs = wpool.tile([P, GMAX_T * (IN + 1)], bf16, name="vsC",
                                tag="vs")
                nc.vector.tensor_tensor(
                    out=O[:, 0:KT * P].rearrange("p (k c) -> p k c", c=P),
                    in0=ccid_sb[:, t0:t0 + KT].rearrange(
                        "p (k one) -> p k one", one=1).to_broadcast(
                        [P, KT, P]),
                    in1=iota_t[:, 0:KT * P].rearrange("p (k c) -> p k c", c=P),
                    op=AO.is_equal)
                vsv = vs[:, 0:KT * (IN + 1)].rearrange(
                    "p (k w) -> p k w", w=IN + 1)
                nc.vector.tensor_tensor(
                    out=vsv[:, :, 0:IN],
                    in0=gsl.rearrange("p (k w) -> p k w", w=TC_W)[:, :, 0:IN],
                    in1=eu[:, 0:KT].rearrange(
                        "p (k one) -> p k one", one=1).to_broadcast(
                        [P, KT, IN]),
                    op=AO.mult)
                nc.scalar.activation(vsv[:, :, IN], eu[:, 0:KT], AF.Copy)
                for s in grp:
                    K = KC[s]
                    b0 = ofs_c[s] - t0
                    ps = psA.tile([P, IN + 1], f32, name="psC_t", tag="psA")
                    for k in range(b0, b0 + K):
                        nc.tensor.matmul(
                            out=ps[:], lhsT=O[:, k * P:(k + 1) * P],
                            rhs=vs[:, k * (IN + 1):(k + 1) * (IN + 1)],
                            start=(k == b0), stop=(k == b0 + K - 1))
                    den = wpool.tile([P, 1], f32, name="denC", tag="den")
                    nc.vector.tensor_scalar_add(den[:], ps[:, IN:IN + 1], 1e-9)
                    rcp = wpool.tile([P, 1], f32, name="rcpC", tag="den")
                    nc.vector.reciprocal(rcp[:], den[:])
                    nc.scalar.activation(
                        osb_res[:, s * IN:(s + 1) * IN], ps[:, 0:IN],
                        AF.Copy, scale=rcp[:, 0:1])
                s = grp[-1]
                # flush finished slots in batches of ~10
                if s + 1 - flushC >= 10 or grp is grpC[-1]:
                    nc.sync.dma_start(
                        out=out_t[:, flushC * IN:(s + 1) * IN],
                        in_=osb_res[:, flushC * IN:(s + 1) * IN])
                    flushC = s + 1

    nc.compile()
    return nc


# ---------------- entry point ----------------

def _make_in_maps(pk):
    in_maps = []
    for c in range(8):
        pc = pk.cores[c]
        g, r = c // 4, c % 4
        m = {
            "xinT": pc.xinT,
            "w_ta": pk.w_ta,
            "w_tb0": pk.w_tb[0], "w_tb1": pk.w_tb[1],
            "a_bc0": pk.a_bc[0], "a_bc1": pk.a_bc[1],
            "w_tc": pk.w_tc,
            "hl": pc.hl,
            "iota_t": pk.iota_t,
            "a_sidx": pc.A.sidx, "a_cidx": pc.A.cidx, "a_alpha": pc.A.alpha,
            "b_sidx0": pc.B.sidx0, "b_sidx1": pc.B.sidx1,
            "b_cidx": pc.B.cidx, "b_cbc": pc.B.cbc,
            "b_ue0": pc.B.streams[0], "b_ue1": pc.B.streams[1],
            "c_sidx": pc.C.sidx, "c_cidx": pc.C.cidx,
            "c_uce": pc.C.streams[0],
        }
        in_maps.append({k: np.ascontiguousarray(v) for k, v in m.items()})
    return in_maps


def kernel(**inputs):
    from concourse.bass_utils import run_bass_kernel_spmd

    pk = _host_prep(inputs)
    nc = _build(pk)
    in_maps = _make_in_maps(pk)
    res = run_bass_kernel_spmd(nc, in_maps, core_ids=list(range(8)))

    x = np.asarray(inputs["x"], np.float32)
    out = np.zeros((BS, ERA, IN), np.float32)
    for g in range(BS):
        quarter = []
        for r in range(4):
            o = np.asarray(res.results[g * 4 + r]["out"]).astype(np.float32)
            quarter.append(o.reshape(P, QBE, IN).transpose(1, 0, 2).reshape(
                QBE * P, IN))
        full = np.concatenate(quarter, 0)[:ERA]
        out[g] = full + x[g, :, :IN]
    return out


# revision 64
# speedup vs baseline: 2.1091x; 1.0569x over previous
"""Trainium2 Bass kernel for nn_MixedTransformer (GNN encode-process-decode).

Distribution: 8 cores = 2 batch groups x 4 dst-range quarters.
Per core: dense val-table matmul over the *used* grid rows only, edge gathers
via dma_gather (bf16 tables), segment-softmax message passing via one-hot
matmuls into PSUM, GAT processor with bf16 table all-gathers inside each
4-core group, decoder back to the grid.

Self-contained: hardcodes all shapes; host does edge sorting/packing and the
encoder's softmax weights (all inputs to that stage are host-visible).
"""
import sys

try:
    import concourse  # noqa: F401
except ImportError:
    sys.path.insert(0, "/opt/trn_rl_repo")

import numpy as np

# ---------------- problem constants ----------------
P = 128
BS = 2
ERA, HMESH = 35718, 10242
IN, AUX, POS = 96, 2, 4
HID, HEADS, DH = 256, 2, 128
E_E2H, E_H2H, E_H2E = 107154, 61440, 107154

ERA_PAD, NBE = 35840, 280          # padded grid rows / dst blocks
MH_PAD, NBM = 10752, 84            # padded mesh rows / dst blocks
QBM, QBE = 21, 70                  # dst blocks per quarter (mesh / grid)
QROWS = QBM * P                    # mesh rows per quarter

TA_W = 256                         # T_A row: val(256) bf16
TB_W = 256                         # T_l row: q(256) bf16
TC_W = 128                         # T_C row: val(96) uS(1) pad(31) bf16

GMAX_T = 8                         # dma_gather hard limit: 1024 indices

RG = [[0, 1, 2, 3], [4, 5, 6, 7]]


# ---------------- host-side packing ----------------

def _seg_softmax_host(logits, seg, n):
    """Exact reference segment softmax (f64), returns per-edge alpha."""
    lg = logits.astype(np.float64)
    m = np.full(n, -np.inf)
    np.maximum.at(m, seg, lg)
    e = np.exp(lg - m[seg])
    s = np.zeros(n)
    np.add.at(s, seg, e)
    return (e / (s[seg] + 1e-9)).astype(np.float64)


def _block_partition(src, dst, nblocks, qb):
    """Group edges by 128-row dst block; per program slot s (0..qb-1) compute
    uniform tile counts K (max over the 4 quarters)."""
    blk = dst // P
    order = np.argsort(blk, kind="stable")
    bo = blk[order]
    starts = np.searchsorted(bo, np.arange(nblocks + 1))
    per_block = [order[starts[j]:starts[j + 1]] for j in range(nblocks)]
    K = [max(-(-len(per_block[qb * r + s]) // P) for r in range(4))
         for s in range(qb)]
    # true max edge count per slot across the 4 quarters (descriptor skip)
    N16 = [max(len(per_block[qb * r + s]) for r in range(4))
           for s in range(qb)]
    return per_block, K, N16


def _chunk_slots(K, budget):
    """Greedily group slot indices so each chunk's tile total <= budget."""
    chunks, cur, tot = [], [], 0
    for s, k in enumerate(K):
        if cur and tot + k > budget:
            chunks.append(cur)
            cur, tot = [], 0
        cur.append(s)
        tot += k
    if cur:
        chunks.append(cur)
    return chunks


def _wrap_idx16(idx_flat):
    """Pack int indices for dma_gather: idx j -> [j%16, j//16], tiled to 128
    partitions. idx_flat length must be a multiple of 128."""
    n = len(idx_flat)
    cols = n // 16
    arr = np.zeros((16, cols), np.int16)
    arr[np.arange(n) % 16, np.arange(n) // 16] = idx_flat
    return np.tile(arr, (8, 1))


def _pad_to(arr, n, fill):
    out = np.full(n, fill, arr.dtype)
    out[:len(arr)] = arr
    return out


class _Packed:
    pass


def _bf16():
    import concourse.mybir as mybir
    return np.dtype(mybir.dt.np(mybir.dt.bfloat16))


def _host_prep(inputs):
    f32 = np.float32
    bf16 = _bf16()
    x = np.asarray(inputs["x"], f32)
    e2h = np.asarray(inputs["e2h_idx"]).astype(np.int64)
    h2h = np.asarray(inputs["h2h_idx"]).astype(np.int64)
    h2e = np.asarray(inputs["h2e_idx"]).astype(np.int64)
    e2h_attr = np.asarray(inputs["e2h_attr"], f32)
    h2h_attr = np.asarray(inputs["h2h_attr"], f32)
    h2e_attr = np.asarray(inputs["h2e_attr"], f32)
    era_ll = np.asarray(inputs["era_latlons"], f32)
    h_ll = np.asarray(inputs["h_latlons"], f32)
    fm_ctx = np.asarray(inputs["fm_ctx"], f32)
    fm_Wsrc = np.asarray(inputs["fm_Wsrc"], f32)
    fm_Wctx = np.asarray(inputs["fm_Wctx"], f32)
    fm_Wedge = np.asarray(inputs["fm_Wedge"], f32)
    fm_att = np.asarray(inputs["fm_att"], f32)
    fm_Wval = np.asarray(inputs["fm_Wval"], f32)
    bm_ctx = np.asarray(inputs["bm_ctx"], f32)
    bm_Wsrc = np.asarray(inputs["bm_Wsrc"], f32)
    bm_Wctx = np.asarray(inputs["bm_Wctx"], f32)
    bm_Wedge = np.asarray(inputs["bm_Wedge"], f32)
    bm_att = np.asarray(inputs["bm_att"], f32)
    bm_Wval = np.asarray(inputs["bm_Wval"], f32)
    gat_W = np.asarray(inputs["gat_W"], f32)
    gat_We = np.asarray(inputs["gat_We"], f32)
    gat_asrc = np.asarray(inputs["gat_asrc"], f32)
    gat_adst = np.asarray(inputs["gat_adst"], f32)
    gat_aedge = np.asarray(inputs["gat_aedge"], f32)

    pk = _Packed()

    # ---- encoder (stage A): host computes exact per-edge alpha ----
    sA, dA = e2h[0], e2h[1]
    x_in = [np.concatenate([x[g].reshape(ERA, IN + AUX), era_ll], 1)
            for g in range(BS)]                                   # (35718,102)
    fm_w_att = fm_Wsrc @ fm_att                                   # (102,)
    uC_A = np.concatenate([fm_ctx, h_ll], 1) @ (fm_Wctx @ fm_att)  # (HMESH,)
    uE_A = e2h_attr @ (fm_Wedge @ fm_att)                         # (E,)
    alphas_A = []
    for g in range(BS):
        uS = x_in[g] @ fm_w_att                                   # (ERA,)
        logit = uS[sA] + uC_A[dA] + uE_A
        lrelu = np.where(logit >= 0, logit, 0.2 * logit)
        alphas_A.append(_seg_softmax_host(lrelu, dA, HMESH))

    pbA, KA, NA16 = _block_partition(sA, dA, NBM, QBM)

    # unique used src rows per quarter (shared across both batch groups)
    U = []
    for r in range(4):
        e = np.concatenate([pbA[QBM * r + s] for s in range(QBM)])
        U.append(np.unique(sA[e]))
    NBU = max(-(-len(u) // P) for u in U)

    # ---- processor (stage B) ----
    sB, dB = h2h[0], h2h[1]
    pbB, KB_, NB16_ = _block_partition(sB, dB, NBM, QBM)
    # split each block's edges by source half-of-quarter (chunked AG overlap)
    halfB = (sB % 2688) >= 1408
    pbB0 = [e[~halfB[e]] for e in pbB]
    pbB1 = [e[halfB[e]] for e in pbB]
    KB0 = [max(-(-len(pbB0[QBM * r + s]) // P) for r in range(4))
           for s in range(QBM)]
    KB1 = [max(-(-len(pbB1[QBM * r + s]) // P) for r in range(4))
           for s in range(QBM)]
    NB0 = [max(len(pbB0[QBM * r + s]) for r in range(4)) for s in range(QBM)]
    NB1 = [max(len(pbB1[QBM * r + s]) for r in range(4)) for s in range(QBM)]
    KB = [KB0[s] + KB1[s] for s in range(QBM)]
    pk.KB0, pk.KB1, pk.NB0, pk.NB1 = KB0, KB1, NB0, NB1
    uE_B = [h2h_attr @ np.einsum("fhd,hd->fh", gat_We[l], gat_aedge[l])
            for l in range(2)]                                    # (E,2)
    w_d = [np.einsum("fhd,hd->fh", gat_W[l], gat_adst[l]) for l in range(2)]
    # fold weights: q(256) + ud(2)
    pk.w_tb = [np.concatenate(
        [gat_W[l].reshape(HID, HID), w_d[l]], 1).astype(bf16) for l in range(2)]
    # per-edge uS = q . a_src, computed on device from gathered q.
    # physically replicated so the batched multiply needs no broadcast AP
    KMB_ = max(KB)
    pk.a_bc = [np.tile(gat_asrc[l].reshape(1, HID), (P, KMB_)).astype(bf16)
               for l in range(2)]                                 # (128,KMB*256)

    # ---- decoder (stage C) ----
    sC, dC = h2e[0], h2e[1]
    pbC, KC, NC16 = _block_partition(sC, dC, NBE, QBE)
    bm_w_att = bm_Wsrc @ bm_att                                   # (260,)
    uC_C = np.concatenate([bm_ctx, era_ll], 1) @ (bm_Wctx @ bm_att)  # (ERA,)
    uE_C = h2e_attr @ (bm_Wedge @ bm_att)                         # (E,)
    uCE_C = uC_C[dC] + uE_C

    pk.w_tc = np.concatenate(
        [bm_Wval[:HID], bm_w_att[:HID, None]], 1).astype(bf16)    # (256,97)
    hl_term = h_ll @ np.concatenate(
        [bm_Wval[HID:], bm_w_att[HID:, None]], 1)                 # (HMESH,97)
    hl_pad = np.zeros((MH_PAD, IN + 1), f32)
    hl_pad[:HMESH] = hl_term
    # per-core bias stream, partition-major: hl[p, s*97+c] = hl_pad[r*2688+s*128+p, c]
    pk.hl = []
    for r in range(4):
        q = hl_pad[QROWS * r:QROWS * (r + 1)].reshape(QBM, P, IN + 1)
        pk.hl.append(np.ascontiguousarray(
            q.transpose(1, 0, 2).reshape(P, QBM * (IN + 1))).astype(bf16))

    pk.w_ta = fm_Wval.astype(bf16)                                # (102,256)

    pk.KA, pk.KB, pk.KC = KA, KB, KC
    pk.NA16, pk.NC16 = NA16, NC16
    SKA, SKB, SKC = sum(KA), sum(KB), sum(KC)
    pk.SKA, pk.SKB, pk.SKC = SKA, SKB, SKC
    pk.NBU = NBU
    pk.KMA, pk.KMB, pk.KMC = max(KA), max(KB), max(KC)
    # stage-C slot groups (shared gather per group, <= 8 tiles)
    grpC, cur, tot = [], [], 0
    for s in range(QBE):
        if cur and tot + KC[s] > 8:
            grpC.append(cur)
            cur, tot = [], 0
        cur.append(s)
        tot += KC[s]
    if cur:
        grpC.append(cur)
    pk.grpC = grpC
    last_of_group = set(g[-1] for g in grpC)
    pk.KMAX = max(pk.KMA, pk.KMB, pk.KMC)

    # iota row template replicated for up to KMAX tiles
    iota_t = np.tile(np.arange(P, dtype=f32), pk.KMAX)
    pk.iota_t = np.tile(iota_t[None, :], (P, 1)).astype(bf16)     # (128,KMAX*128)

    def pack_quarter_A(r, g):
        sidx, cidx, alph = [], [], []
        remap = np.full(ERA, 0, np.int64)
        remap[U[r]] = np.arange(len(U[r]))
        for s in range(QBM):
            j = QBM * r + s
            e = pbA[j]
            n = KA[s] * P
            si = np.full(n, -1, np.int16)
            si[:NA16[s]] = 0
            si[:len(e)] = remap[sA[e]].astype(np.int16)
            sidx.append(si)
            cidx.append(_pad_to((dA[e] - j * P).astype(f32), n, -1.0))
            alph.append(_pad_to(alphas_A[g][e].astype(f32), n, 0.0))
        out = _Packed()
        out.sidx = _wrap_idx16(np.concatenate(sidx))
        out.cidx = np.concatenate(cidx).reshape(SKA, P).T.copy()
        out.alpha = np.concatenate(alph).reshape(SKA, P).T.copy()
        return out

    def pack_quarter_BC(r, per_block, K, qb, src, dst, SK, streams, cbc,
                        nmax, tail_neg=None):
        """streams: list of per-edge arrays (E,) or (E,m) -> packed (128, SK*m)."""
        sidx, cidx, st_out = [], [], [[] for _ in streams]
        for s in range(qb):
            j = qb * r + s
            e = per_block[j]
            n = K[s] * P
            if tail_neg is None or s in tail_neg:
                si = np.full(n, -1, np.int16)
                si[:nmax[s]] = 0
            else:
                si = np.zeros(n, np.int16)
            si[:len(e)] = src[e].astype(np.int16)
            sidx.append(si)
            cidx.append(_pad_to((dst[e] - j * P).astype(f32), n, -1.0))
            for q, arr in enumerate(streams):
                a = arr[e]
                if a.ndim == 1:
                    a = a[:, None]
                m = a.shape[1]
                buf = np.zeros((n, m), f32)
                buf[:len(e)] = a
                st_out[q].append(buf)
        out = _Packed()
        out.sidx = _wrap_idx16(np.concatenate(sidx))
        cf = np.concatenate(cidx).reshape(SK, P).T
        out.cidx = cf.copy()
        if cbc:
            # broadcast cidx: [p, t*128+c] = cidx of edge (t, c), same all p
            out.cbc = np.ascontiguousarray(
                np.tile(cf.T.reshape(1, SK * P), (P, 1))).astype(bf16)
        out.streams = []
        for q, parts in enumerate(st_out):
            a = np.concatenate(parts, 0)                          # (SK*P, m)
            m = a.shape[1]
            out.streams.append(
                a.reshape(SK, P, m).transpose(1, 0, 2).reshape(P, SK * m).copy())
        return out

    def pack_B(r):
        sidx0, sidx1, cidx, ues = [], [], [], [[], []]
        for s in range(QBM):
            j = QBM * r + s
            for half, (pb, Kh, Nh, sil, rows, off) in enumerate(
                    [(pbB0, KB0, NB0, sidx0, 1408, 0),
                     (pbB1, KB1, NB1, sidx1, 1280, 1408)]):
                e = pb[j]
                n = Kh[s] * P
                if n == 0:
                    continue
                m = sB[e]
                rem = (m // 2688) * rows + (m % 2688) - off
                si = np.full(n, -1, np.int16)
                si[:Nh[s]] = 0
                si[:len(e)] = rem.astype(np.int16)
                sil.append(si)
                cidx.append(_pad_to((dB[e] - j * P).astype(f32), n, -1.0))
                for l in range(2):
                    buf = np.zeros((n, 2), f32)
                    buf[:len(e)] = uE_B[l][e]
                    ues[l].append(buf)
        out = _Packed()
        out.sidx0 = _wrap_idx16(np.concatenate(sidx0))
        out.sidx1 = _wrap_idx16(np.concatenate(sidx1))
        cf = np.concatenate(cidx).reshape(SKB, P).T
        out.cidx = cf.copy()
        out.cbc = np.ascontiguousarray(
            np.tile(cf.T.reshape(1, SKB * P), (P, 1))).astype(bf16)
        out.streams = []
        for l in range(2):
            a = np.concatenate(ues[l], 0)
            out.streams.append(
                a.reshape(SKB, P, 2).transpose(1, 0, 2).reshape(
                    P, SKB * 2).copy())
        return out

    pk.cores = []
    for c in range(8):
        g, r = c // 4, c % 4
        pc = _Packed()
        pc.A = pack_quarter_A(r, g)
        pc.B = pack_B(r)
        pc.C = pack_quarter_BC(r, pbC, KC, QBE, sC, dC, SKC,
                               [uCE_C], cbc=False, nmax=NC16,
                               tail_neg=last_of_group)
        pc.hl = pk.hl[r]
        # compact transposed encoder input, feature-major: (102, NBU*128)
        xt = np.zeros((IN + AUX + POS, NBU * P), f32)
        xt[:, :len(U[r])] = x_in[g][U[r]].T
        pc.xinT = xt.astype(bf16)
        pk.cores.append(pc)
    return pk


# ---------------- device program ----------------

def _build(pk):
    import concourse.mybir as mybir
    import concourse.tile as tile
    from concourse import bacc
    from concourse.masks import make_identity

    f32 = mybir.dt.float32
    bf16 = mybir.dt.bfloat16
    i16 = mybir.dt.int16
    AO = mybir.AluOpType
    AF = mybir.ActivationFunctionType

    nc = bacc.Bacc("TRN2", target_bir_lowering=False, debug=False,
                   num_devices=8)

    SKA, SKB, SKC = pk.SKA, pk.SKB, pk.SKC
    KA, KB, KC = pk.KA, pk.KB, pk.KC
    NA16, NC16 = pk.NA16, pk.NC16
    KB0, KB1, NB0, NB1 = pk.KB0, pk.KB1, pk.NB0, pk.NB1
    NBU, KMAX = pk.NBU, pk.KMAX
    KMA, KMB, KMC = pk.KMA, pk.KMB, pk.KMC
    IN_F = IN + AUX + POS
    ein = {}

    def xin(name, shape, dt=bf16):
        ein[name] = nc.dram_tensor(name, shape, dt, kind="ExternalInput")
        return ein[name]

    xinT = xin("xinT", [IN_F, NBU * P])
    w_ta = xin("w_ta", [IN_F, TA_W])
    w_tb0 = xin("w_tb0", [HID, HID + 2])
    w_tb1 = xin("w_tb1", [HID, HID + 2])
    a_bc0 = xin("a_bc0", [P, KMB * HID])
    a_bc1 = xin("a_bc1", [P, KMB * HID])
    w_tc = xin("w_tc", [HID, IN + 1])
    hl = xin("hl", [P, QBM * (IN + 1)])
    iota_in = xin("iota_t", [P, KMAX * P])
    a_sidx = xin("a_sidx", [P, SKA * 8], i16)
    a_cidx = xin("a_cidx", [P, SKA], f32)
    a_alpha = xin("a_alpha", [P, SKA], f32)
    SKB0, SKB1 = sum(pk.KB0), sum(pk.KB1)
    b_sidx0 = xin("b_sidx0", [P, SKB0 * 8], i16)
    b_sidx1 = xin("b_sidx1", [P, SKB1 * 8], i16)
    b_cidx = xin("b_cidx", [P, SKB], f32)
    b_cbc = xin("b_cbc", [P, SKB * P])
    b_ue0 = xin("b_ue0", [P, SKB * 2], f32)
    b_ue1 = xin("b_ue1", [P, SKB * 2], f32)
    c_sidx = xin("c_sidx", [P, SKC * 8], i16)
    c_cidx = xin("c_cidx", [P, SKC], f32)
    c_uce = xin("c_uce", [P, SKC], f32)
    out_t = nc.dram_tensor("out", [P, QBE * IN], bf16, kind="ExternalOutput")
    import os
    _lvl = int(os.environ.get("KERNEL_PHASES", "8"))
    _sub = int(os.environ.get("KERNEL_SUB", "4"))

    ofs_a = np.cumsum([0] + KA)
    ofs_b = np.cumsum([0] + KB)
    ofs_c = np.cumsum([0] + KC)

    with tile.TileContext(nc) as tc:
        with tc.tile_pool(name="const", bufs=1) as cpool, \
             tc.tile_pool(name="stream", bufs=1) as spool, \
             tc.tile_pool(name="res", bufs=1) as rpool, \
             tc.tile_pool(name="gat", bufs=3) as gpool, \
             tc.tile_pool(name="lx", bufs=2) as lxpool, \
             tc.tile_pool(name="work", bufs=3) as wpool, \
             tc.tile_pool(name="ob", bufs=2) as obpool, \
             tc.tile_pool(name="psA", bufs=2, space="PSUM") as psA, \
             tc.tile_pool(name="psU", bufs=2, space="PSUM") as psU, \
             tc.tile_pool(name="psT", bufs=2, space="PSUM") as psT, \
             tc.tile_pool(name="psF", bufs=2, space="PSUM") as psF, \
             tc.tile_pool(name="dram", bufs=1, space="DRAM") as dpool:

            # ---------- constants / streams ----------
            ident = cpool.tile([P, P], bf16, name="ident")
            make_identity(nc, ident[:])

            def load(name, src, shape, dt=bf16):
                t = spool.tile(shape, dt, name=name)
                nc.sync.dma_start(out=t[:], in_=src[tuple(slice(0, s) for s in shape)])
                return t

            w_ta_sb = load("w_ta_sb", w_ta, [IN_F, TA_W])

            def load_half(name, src, h, cols):
                t = spool.tile([P, cols], bf16, name=name)
                nc.sync.dma_start(out=t[:], in_=src[h * P:(h + 1) * P, 0:cols])
                return t[:]

            w_tb_sb = [[load_half(f"w_tb{l}_{h}", [w_tb0, w_tb1][l], h, HID + 2)
                        for h in range(2)] for l in range(2)]
            a_bc_sb = [load(f"a_bc{l}_sb", [a_bc0, a_bc1][l], [P, KMB * HID])
                       for l in range(2)]
            w_tc_sb = [load_half(f"w_tc_{h}", w_tc, h, IN + 1)
                       for h in range(2)]
            hl_sb = load("hl_sb", hl, [P, QBM * (IN + 1)])
            iota_t = load("iota_sb", iota_in, [P, KMAX * P])
            asid_sb = load("asid_sb", a_sidx, [P, SKA * 8], i16)
            acid_sb = load("acid_sb", a_cidx, [P, SKA], f32)
            aal_sb = load("aal_sb", a_alpha, [P, SKA], f32)
            bsid0_sb = load("bsid0_sb", b_sidx0, [P, SKB0 * 8], i16)
            bsid1_sb = load("bsid1_sb", b_sidx1, [P, SKB1 * 8], i16)
            bcid_sb = load("bcid_sb", b_cidx, [P, SKB], f32)
            bcbc_sb = load("bcbc_sb", b_cbc, [P, SKB * P])
            bue_sb = [load("bue0_sb", b_ue0, [P, SKB * 2], f32),
                      load("bue1_sb", b_ue1, [P, SKB * 2], f32)]
            csid_sb = load("csid_sb", c_sidx, [P, SKC * 8], i16)
            ccid_sb = load("ccid_sb", c_cidx, [P, SKC], f32)
            cuce_sb = load("cuce_sb", c_uce, [P, SKC], f32)

            # partition-index column (value = partition id), bf16
            piota_i = cpool.tile([P, 1], mybir.dt.int32, name="piota_i")
            nc.gpsimd.iota(piota_i[:], pattern=[[1, 1]], base=0,
                           channel_multiplier=1)
            piota = cpool.tile([P, 1], bf16, name="piota")
            nc.vector.tensor_copy(piota[:], piota_i[:])

            # persistent gather buffers (manual round-robin, zeroed once
            # so descriptor-skipped pad rows stay finite)
            gbufs = [rpool.tile([P, KMA * TA_W], bf16, name=f"gbuf{i}")
                     for i in range(5)]
            for t in gbufs:
                nc.vector.memset(t[:], 0.0)
            gb_i = [0]

            def next_gb():
                t = gbufs[gb_i[0] % 5]
                gb_i[0] += 1
                return t

            # ---------- resident quarter features (bf16) ----------
            xlat = rpool.tile([P, QBM * HID], bf16, name="xlat")
            h1g = rpool.tile([P, QBM * HID], bf16, name="h1g")
            xproc = rpool.tile([P, QBM * HID], bf16, name="xproc")
            ud_res = [rpool.tile([P, QBM * 2], bf16, name=f"ud{l}")
                      for l in range(2)]
            osb_res = rpool.tile([P, QBE * IN], bf16, name="osb")

            # ---------- DRAM tables (bf16) ----------
            ta_dram = dpool.tile([NBU * P, TA_W], bf16, name="ta_dram")
            HR0, HR1 = 11 * P, 10 * P
            tb_locA = [dpool.tile([HR0, TB_W], bf16, name=f"tb_locA{l}")
                       for l in range(2)]
            tb_locB = [dpool.tile([HR1, TB_W], bf16, name=f"tb_locB{l}")
                       for l in range(2)]
            tb_halfA = [dpool.tile([4 * HR0, TB_W], bf16, name=f"tb_hA{l}")
                        for l in range(2)]
            tb_halfB = [dpool.tile([4 * HR1, TB_W], bf16, name=f"tb_hB{l}")
                        for l in range(2)]
            tc_loc = dpool.tile([QROWS, TC_W], bf16, name="tc_loc")
            tc_full = dpool.tile([MH_PAD, TC_W], bf16, name="tc_full")

            # ---------- phase 1: dense T_A (compact rows) ----------
            CW = 16                               # blocks per input chunk
            for c0 in range(0, NBU if _lvl >= 1 else 0, CW):
                cw = min(CW, NBU - c0)
                lx = lxpool.tile([IN_F, CW * P], bf16, name="lx", tag="lx")
                nc.sync.dma_start(out=lx[:, 0:cw * P],
                                  in_=xinT[:, c0 * P:(c0 + cw) * P])
                for b0 in range(0, cw, 8):
                    bw = min(8, cw - b0)
                    ta_sb = wpool.tile([P, 8 * TA_W], bf16, name="ta_sb",
                                       tag="ta_sb")
                    for k in range(bw):
                        pst = psT.tile([P, TA_W], f32, name="ps_ta", tag="pst")
                        nc.tensor.matmul(
                            out=pst[:],
                            lhsT=lx[:, (b0 + k) * P:(b0 + k + 1) * P],
                            rhs=w_ta_sb[:], start=True, stop=True)
                        nc.scalar.activation(
                            ta_sb[:, k * TA_W:(k + 1) * TA_W], pst[:], AF.Copy)
                    base = (c0 + b0) * P
                    nc.sync.dma_start(
                        out=ta_dram[base:base + bw * P, :].rearrange(
                            "(b p) w -> p b w", p=P),
                        in_=ta_sb[:, 0:bw * TA_W].rearrange(
                            "p (b w) -> p b w", w=TA_W))

            # ---------- helper: one-hot for tile k of a slot ----
            def onehot(dst_ap, cidx_col):
                # dst[p, j] = (cidx[p] == j)
                nc.vector.tensor_tensor(
                    out=dst_ap, in0=cidx_col.to_broadcast([P, P]),
                    in1=iota_t[:, 0:P], op=AO.is_equal)

            # ---------- phase 2: stage A (encoder edges) ----------
            for s in range(QBM if _lvl >= 2 else 0):
                K = KA[s]
                if K == 0:
                    continue
                t0 = ofs_a[s]
                gb = next_gb()
                for c0 in range(0, K, GMAX_T):
                    cw = min(GMAX_T, K - c0)
                    nfull = cw * P
                    nreg = min(max(NA16[s] - c0 * P, 0), nfull)
                    nc.gpsimd.dma_gather(
                        out_ap=gb[:, c0 * TA_W:(c0 + cw) * TA_W].rearrange(
                            "p (k w) -> p k w", w=TA_W),
                        in_ap=ta_dram[:, :],
                        idxs_ap=asid_sb[:, (t0 + c0) * 8:(t0 + c0 + cw) * 8],
                        num_idxs=nfull, num_idxs_reg=nreg,
                        elem_size=TA_W, single_packet=False)
                if True:
                    if _sub < 2:
                        continue
                    S = obpool.tile([P, KMA * P], bf16, name="S_A", tag="ob")
                    S3 = S[:, 0:K * P].rearrange("p (k c) -> p k c", c=P)
                    nc.vector.tensor_tensor(
                        out=S3,
                        in0=acid_sb[:, t0:t0 + K].rearrange(
                            "p (k one) -> p k one", one=1).to_broadcast(
                            [P, K, P]),
                        in1=iota_t[:, 0:K * P].rearrange(
                            "p (k c) -> p k c", c=P),
                        op=AO.is_equal)
                    nc.vector.tensor_tensor(
                        out=S3, in0=S3,
                        in1=aal_sb[:, t0:t0 + K].rearrange(
                            "p (k one) -> p k one", one=1).to_broadcast(
                            [P, K, P]),
                        op=AO.mult)
                    if _sub < 3:
                        continue
                    ps = psA.tile([P, HID], f32, name="psA_t", tag="psA")
                    for k in range(K):
                        nc.tensor.matmul(
                            out=ps[:], lhsT=S[:, k * P:(k + 1) * P],
                            rhs=gb[:, k * TA_W:k * TA_W + HID],
                            start=(k == 0), stop=(k == K - 1))
                    if _sub < 4:
                        continue
                    nc.scalar.activation(
                        xlat[:, s * HID:(s + 1) * HID], ps[:], AF.Copy)

            # ---------- helper: fold resident -> table ----------
            def fold_slot(src, w_sb, wcols, s, dst_dram=None, ud_dst=None,
                          bias_sb=None, dst_half=None):
                if True:
                    pst = psT.tile([P, HID], bf16, name="ps_tr", tag="pst")
                    for h in range(2):
                        nc.tensor.transpose(
                            out=pst[:, h * P:(h + 1) * P],
                            in_=src[:, s * HID + h * P:s * HID + (h + 1) * P],
                            identity=ident[:])
                    xt = wpool.tile([P, HID], bf16, name="xt", tag="xt")
                    nc.vector.tensor_copy(xt[:], pst[:])
                    psf = psF.tile([P, wcols], f32, name="ps_f", tag="psf")
                    for h in range(2):
                        nc.tensor.matmul(out=psf[:],
                                         lhsT=xt[:, h * P:(h + 1) * P],
                                         rhs=w_sb[h][:, 0:wcols],
                                         start=(h == 0), stop=(h == 1))
                    fsb = wpool.tile([P, TB_W], bf16, name="fsb", tag="fsb")
                    if bias_sb is not None:
                        nc.vector.tensor_tensor(
                            out=fsb[:, 0:wcols], in0=psf[:, 0:wcols],
                            in1=bias_sb[:, s * wcols:(s + 1) * wcols],
                            op=AO.add)
                        nc.sync.dma_start(
                            out=dst_dram[s * P:(s + 1) * P, 0:wcols],
                            in_=fsb[:, 0:wcols])
                    else:
                        nc.scalar.activation(fsb[:, 0:HID], psf[:, 0:HID],
                                             AF.Copy)
                        da, db_ = dst_half
                        dap = (da[s * P:(s + 1) * P, 0:HID] if s < 11 else
                               db_[(s - 11) * P:(s - 10) * P, 0:HID])
                        nc.sync.dma_start(out=dap, in_=fsb[:, 0:HID])
                        nc.scalar.activation(
                            ud_dst[:, 2 * s:2 * s + 2], psf[:, HID:HID + 2],
                            AF.Copy)


            def ag_half(loc, full):
                nc.gpsimd.collective_compute(
                    "AllGather", AO.bypass, replica_groups=RG,
                    ins=[loc.opt()], outs=[full.opt()])

            if _lvl >= 3:
                fold(xlat, w_tb_sb[0], HID + 2, None, ud_dst=ud_res[0],
                     dst_half=(tb_locA[0], tb_locB[0]),
                     after10=lambda: ag_half(tb_locA[0], tb_halfA[0]))
                ag_half(tb_locB[0], tb_halfB[0])

            # ---------- phase 3/4: GAT layers ----------
            ofs_b0 = np.cumsum([0] + KB0)
            ofs_b1 = np.cumsum([0] + KB1)

            def gat_layer(l, dst_res, residual, after_slot=None):
                for s in range(QBM):
                    K = KB[s]
                    if K == 0:
                        if after_slot is not None:
                            after_slot(s)
                        continue
                    t0 = ofs_b[s]
                    gb = next_gb()
                    for tfull, Kh, Nh, ofs_h, sid in (
                            (tb_halfA[l], KB0, NB0, ofs_b0, bsid0_sb),
                            (tb_halfB[l], KB1, NB1, ofs_b1, bsid1_sb)):
                        if Kh[s] == 0:
                            continue
                        o0 = 0 if tfull is tb_halfA[l] else KB0[s]
                        h0 = ofs_h[s]
                        nc.gpsimd.dma_gather(
                            out_ap=gb[:, o0 * TB_W:
                                      (o0 + Kh[s]) * TB_W].rearrange(
                                "p (k w) -> p k w", w=TB_W),
                            in_ap=tfull[:, :],
                            idxs_ap=sid[:, h0 * 8:(h0 + Kh[s]) * 8],
                            num_idxs=Kh[s] * P, num_idxs_reg=Nh[s],
                            elem_size=TB_W, single_packet=False)
                    if True:
                        gsl = gb[:, 0:K * TB_W]
                        O = obpool.tile([P, KMB * P], bf16, name="O_B",
                                        tag="ob")
                        nc.vector.tensor_tensor(
                            out=O[:, 0:K * P].rearrange(
                                "p (k c) -> p k c", c=P),
                            in0=bcid_sb[:, t0:t0 + K].rearrange(
                                "p (k one) -> p k one", one=1).to_broadcast(
                                [P, K, P]),
                            in1=iota_t[:, 0:K * P].rearrange(
                                "p (k c) -> p k c", c=P),
                            op=AO.is_equal)
                        OT = wpool.tile([P, KMB * P], bf16, name="OT",
                                        tag="ot")
                        nc.vector.tensor_tensor(
                            out=OT[:, 0:K * P],
                            in0=piota[:].to_broadcast([P, K * P]),
                            in1=bcbc_sb[:, t0 * P:(t0 + K) * P],
                            op=AO.is_equal)
                        # us[p, 2k+h] = sum_d q[p,k,h,d] * a_src[h,d]
                        tmp = wpool.tile([P, KMB * TB_W], bf16, name="tmp",
                                         tag="tmp")
                        nc.vector.tensor_tensor(
                            out=tmp[:, 0:K * TB_W],
                            in0=gsl,
                            in1=a_bc_sb[l][:, 0:K * TB_W], op=AO.mult)
                        us = wpool.tile([P, 2 * KMAX], f32, name="us",
                                        tag="us")
                        nc.vector.tensor_reduce(
                            out=us[:, 0:2 * K],
                            in_=tmp[:, 0:K * TB_W].rearrange(
                                "p (k h d) -> p k h d", h=2, d=DH),
                            axis=mybir.AxisListType.X, op=AO.add)
                        psu = psU.tile([P, 2 * KMAX], f32, name="psu",
                                       tag="psu")
                        for k in range(K):
                            nc.tensor.matmul(
                                out=psu[:, 2 * k:2 * k + 2],
                                lhsT=OT[:, k * P:(k + 1) * P],
                                rhs=ud_res[l][:, 2 * s:2 * s + 2],
                                start=True, stop=True)
                        t2 = wpool.tile([P, 2 * KMAX], f32, name="t2",
                                        tag="eu")
                        nc.vector.tensor_tensor(
                            out=t2[:, 0:2 * K], in0=us[:, 0:2 * K],
                            in1=psu[:, 0:2 * K], op=AO.add)
                        t3 = wpool.tile([P, 2 * KMAX], f32, name="t3",
                                        tag="eu")
                        nc.vector.tensor_tensor(
                            out=t3[:, 0:2 * K], in0=t2[:, 0:2 * K],
                            in1=bue_sb[l][:, t0 * 2:(t0 + K) * 2], op=AO.add)
                        t4 = wpool.tile([P, 2 * KMAX], f32, name="t4",
                                        tag="eu")
                        nc.vector.scalar_tensor_tensor(
                            out=t4[:, 0:2 * K], in0=t3[:, 0:2 * K],
                            scalar=0.2, in1=t3[:, 0:2 * K],
                            op0=AO.mult, op1=AO.max)
                        eu = wpool.tile([P, 2 * KMAX], f32, name="eu",
                                        tag="eu")
                        nc.scalar.activation(eu[:, 0:2 * K], t4[:, 0:2 * K],
                                             AF.Exp)
                        # vs[p, k*258 + h*128+d] = q * eu ; cols 256:258 = eu
                        vs = wpool.tile([P, KMB * (TB_W + 2)], bf16,
                                        name="vs", tag="vs")
                        vsv = vs[:, 0:K * (TB_W + 2)].rearrange(
                            "p (k w) -> p k w", w=TB_W + 2)
                        nc.vector.tensor_tensor(
                            out=vsv[:, :, 0:TB_W].rearrange(
                                "p k (h d) -> p k h d", h=2),
                            in0=gsl.rearrange("p (k h d) -> p k h d",
                                              h=2, d=DH),
                            in1=eu[:, 0:2 * K].rearrange(
                                "p (k h one) -> p k h one",
                                h=2, one=1).to_broadcast([P, K, 2, DH]),
                            op=AO.mult)
                        nc.scalar.activation(
                            vsv[:, :, TB_W:TB_W + 2],
                            eu[:, 0:2 * K].rearrange("p (k h) -> p k h", h=2),
                            AF.Copy)
                        ps = psA.tile([P, TB_W + 2], f32, name="psB_t",
                                      tag="psA")
                        for k in range(K):
                            nc.tensor.matmul(
                                out=ps[:],
                                lhsT=O[:, k * P:(k + 1) * P],
                                rhs=vs[:, k * (TB_W + 2):(k + 1) * (TB_W + 2)],
                                start=(k == 0), stop=(k == K - 1))
                        den = wpool.tile([P, 2], f32, name="den", tag="den")
                        nc.vector.tensor_scalar_add(den[:], ps[:, TB_W:TB_W + 2],
                                                    1e-9)
                        rcp = wpool.tile([P, 2], f32, name="rcp", tag="den")
                        nc.vector.reciprocal(rcp[:], den[:])
                        for h in range(2):
                            od = dst_res[:, s * HID + h * P:s * HID + (h + 1) * P]
                            if residual is None:
                                nc.scalar.activation(
                                    od, ps[:, h * P:(h + 1) * P], AF.Copy,
                                    scale=rcp[:, h:h + 1])
                            else:
                                nc.vector.scalar_tensor_tensor(
                                    out=od, in0=ps[:, h * P:(h + 1) * P],
                                    scalar=rcp[:, h:h + 1],
                                    in1=residual[:, s * HID + h * P:
                                                 s * HID + (h + 1) * P],
                                    op0=AO.mult, op1=AO.add)
                    if after_slot is not None:
                        after_slot(s)

            # layer-1 prep emitted in halves inside gat0: gelu+fold1+AG
            def after_gat0(s):
                if _lvl < 5 or (s != 10 and s != QBM - 1):
                    return
                lo = 0 if s == 10 else 11
                for s2 in range(lo, s + 1):
                    xs = h1g[:, s2 * HID:(s2 + 1) * HID]
                    nc.scalar.activation(xs, xs, AF.Gelu_apprx_tanh)
                for s2 in range(lo, s + 1):
                    fold_slot(h1g, w_tb_sb[1], HID + 2, s2,
                              ud_dst=ud_res[1],
                              dst_half=(tb_locA[1], tb_locB[1]))
                if s == 10:
                    ag_half(tb_locA[1], tb_halfA[1])
                else:
                    ag_half(tb_locB[1], tb_halfB[1])

            if _lvl >= 4:
                gat_layer(0, h1g, None, after_slot=after_gat0)

            def after_gat1(s):
                if _lvl >= 7:
                    fold_slot(xproc, w_tc_sb, IN + 1, s, dst_dram=tc_loc,
                              bias_sb=hl_sb)

            if _lvl >= 6:
                gat_layer(1, xproc, xlat, after_slot=after_gat1)

            if _lvl >= 7:
                nc.gpsimd.collective_compute(
                    "AllGather", AO.bypass, replica_groups=RG,
                    ins=[tc_loc.opt()], outs=[tc_full.opt()])

            # ---------- phase 5: decoder (stage C) ----------
            grpC = pk.grpC
            flushC = 0
            for grp in (grpC if _lvl >= 8 else []):
                t0 = ofs_c[grp[0]]
                KT = ofs_c[grp[-1] + 1] - t0
                nreg = (ofs_c[grp[-1]] - t0) * P + NC16[grp[-1]]
                gb = next_gb()
                nc.gpsimd.dma_gather(
                    out_ap=gb[:, 0:KT * TC_W].rearrange(
                        "p (k w) -> p k w", w=TC_W),
                    in_ap=tc_full[:, :],
                    idxs_ap=csid_sb[:, t0 * 8:(t0 + KT) * 8],
                    num_idxs=KT * P, num_idxs_reg=nreg,
                    elem_size=TC_W, single_packet=False)
                gsl = gb[:, 0:KT * TC_W]
                tt = wpool.tile([P, GMAX_T], f32, name="ttC", tag="eu")
                nc.vector.tensor_tensor(
                    out=tt[:, 0:KT],
                    in0=gsl.rearrange("p (k w) -> p k w",
                                      w=TC_W)[:, :, IN],
                    in1=cuce_sb[:, t0:t0 + KT], op=AO.add)
                t4 = wpool.tile([P, GMAX_T], f32, name="t4C", tag="eu")
                nc.vector.scalar_tensor_tensor(
                    out=t4[:, 0:KT], in0=tt[:, 0:KT], scalar=0.2,
                    in1=tt[:, 0:KT], op0=AO.mult, op1=AO.max)
                eu = wpool.tile([P, GMAX_T], f32, name="euC", tag="eu")
                nc.scalar.activation(eu[:, 0:KT], t4[:, 0:KT], AF.Exp)
                O = obpool.tile([P, GMAX_T * P], bf16, name="O_C", tag="ob")
                vs = wpool.tile([P, GMAX_T * (IN + 1)], bf16, name="vsC",
                                tag="vs")
                nc.vector.tensor_tensor(
                    out=O[:, 0:KT * P].rearrange("p (k c) -> p k c", c=P),
                    in0=ccid_sb[:, t0:t0 + KT].rearrange(
                        "p (k one) -> p k one", one=1).to_broadcast(
                        [P, KT, P]),
                    in1=iota_t[:, 0:KT * P].rearrange("p (k c) -> p k c", c=P),
                    op=AO.is_equal)
                vsv = vs[:, 0:KT * (IN + 1)].rearrange(
                    "p (k w) -> p k w", w=IN + 1)
                nc.vector.tensor_tensor(
                    out=vsv[:, :, 0:IN],
                    in0=gsl.rearrange("p (k w) -> p k w", w=TC_W)[:, :, 0:IN],
                    in1=eu[:, 0:KT].rearrange(
                        "p (k one) -> p k one", one=1).to_broadcast(
                        [P, KT, IN]),
                    op=AO.mult)
                nc.scalar.activation(vsv[:, :, IN], eu[:, 0:KT], AF.Copy)
                for s in grp:
                    K = KC[s]
                    b0 = ofs_c[s] - t0
                    ps = psA.tile([P, IN + 1], f32, name="psC_t", tag="psA")
                    for k in range(b0, b0 + K):
                        nc.tensor.matmul(
                            out=ps[:], lhsT=O[:, k * P:(k + 1) * P],
                            rhs=vs[:, k * (IN + 1):(k + 1) * (IN + 1)],
                            start=(k == b0), stop=(k == b0 + K - 1))
                    den = wpool.tile([P, 1], f32, name="denC", tag="den")
                    nc.vector.tensor_scalar_add(den[:], ps[:, IN:IN + 1], 1e-9)
                    rcp = wpool.tile([P, 1], f32, name="rcpC", tag="den")
                    nc.vector.reciprocal(rcp[:], den[:])
                    nc.scalar.activation(
                        osb_res[:, s * IN:(s + 1) * IN], ps[:, 0:IN],
                        AF.Copy, scale=rcp[:, 0:1])
                s = grp[-1]
                # flush finished slots in batches of ~10
                if s + 1 - flushC >= 10 or grp is grpC[-1]:
                    nc.sync.dma_start(
                        out=out_t[:, flushC * IN:(s + 1) * IN],
                        in_=osb_res[:, flushC * IN:(s + 1) * IN])
                    flushC = s + 1

    nc.compile()
    return nc


# ---------------- entry point ----------------

def _make_in_maps(pk):
    in_maps = []
    for c in range(8):
        pc = pk.cores[c]
        g, r = c // 4, c % 4
        m = {
            "xinT": pc.xinT,
            "w_ta": pk.w_ta,
            "w_tb0": pk.w_tb[0], "w_tb1": pk.w_tb[1],
            "a_bc0": pk.a_bc[0], "a_bc1": pk.a_bc[1],
            "w_tc": pk.w_tc,
            "hl": pc.hl,
            "iota_t": pk.iota_t,
            "a_sidx": pc.A.sidx, "a_cidx": pc.A.cidx, "a_alpha": pc.A.alpha,
            "b_sidx0": pc.B.sidx0, "b_sidx1": pc.B.sidx1,
            "b_cidx": pc.B.cidx, "b_cbc": pc.B.cbc,
            "b_ue0": pc.B.streams[0], "b_ue1": pc.B.streams[1],
            "c_sidx": pc.C.sidx, "c_cidx": pc.C.cidx,
            "c_uce": pc.C.streams[0],
        }
        in_maps.append({k: np.ascontiguousarray(v) for k, v in m.items()})
    return in_maps


def kernel(**inputs):
    from concourse.bass_utils import run_bass_kernel_spmd

    pk = _host_prep(inputs)
    nc = _build(pk)
    in_maps = _make_in_maps(pk)
    res = run_bass_kernel_spmd(nc, in_maps, core_ids=list(range(8)))

    x = np.asarray(inputs["x"], np.float32)
    out = np.zeros((BS, ERA, IN), np.float32)
    for g in range(BS):
        quarter = []
        for r in range(4):
            o = np.asarray(res.results[g * 4 + r]["out"]).astype(np.float32)
            quarter.append(o.reshape(P, QBE, IN).transpose(1, 0, 2).reshape(
                QBE * P, IN))
        full = np.concatenate(quarter, 0)[:ERA]
        out[g] = full + x[g, :, :IN]
    return out


# revision 65
# speedup vs baseline: 2.6979x; 1.2792x over previous
"""Trainium2 Bass kernel for nn_MixedTransformer (GNN encode-process-decode).

Distribution: 8 cores = 2 batch groups x 4 dst-range quarters.
Per core: dense val-table matmul over the *used* grid rows only, edge gathers
via dma_gather (bf16 tables), segment-softmax message passing via one-hot
matmuls into PSUM, GAT processor with bf16 table all-gathers inside each
4-core group, decoder back to the grid.

Self-contained: hardcodes all shapes; host does edge sorting/packing and the
encoder's softmax weights (all inputs to that stage are host-visible).
"""
import sys

try:
    import concourse  # noqa: F401
except ImportError:
    sys.path.insert(0, "/opt/trn_rl_repo")

import numpy as np

# ---------------- problem constants ----------------
P = 128
BS = 2
ERA, HMESH = 35718, 10242
IN, AUX, POS = 96, 2, 4
HID, HEADS, DH = 256, 2, 128
E_E2H, E_H2H, E_H2E = 107154, 61440, 107154

ERA_PAD, NBE = 35840, 280          # padded grid rows / dst blocks
MH_PAD, NBM = 10752, 84            # padded mesh rows / dst blocks
QBM, QBE = 21, 70                  # dst blocks per quarter (mesh / grid)
QROWS = QBM * P                    # mesh rows per quarter

TA_W = 256                         # T_A row: val(256) bf16
TB_W = 256                         # T_l row: q(256) bf16
TC_W = 128                         # T_C row: val(96) uS(1) pad(31) bf16

GMAX_T = 8                         # dma_gather hard limit: 1024 indices

RG = [[0, 1, 2, 3], [4, 5, 6, 7]]


# ---------------- host-side packing ----------------

def _seg_softmax_host(logits, seg, n):
    """Exact reference segment softmax (f64), returns per-edge alpha."""
    lg = logits.astype(np.float64)
    m = np.full(n, -np.inf)
    np.maximum.at(m, seg, lg)
    e = np.exp(lg - m[seg])
    s = np.zeros(n)
    np.add.at(s, seg, e)
    return (e / (s[seg] + 1e-9)).astype(np.float64)


def _block_partition(src, dst, nblocks, qb):
    """Group edges by 128-row dst block; per program slot s (0..qb-1) compute
    uniform tile counts K (max over the 4 quarters)."""
    blk = dst // P
    order = np.argsort(blk, kind="stable")
    bo = blk[order]
    starts = np.searchsorted(bo, np.arange(nblocks + 1))
    per_block = [order[starts[j]:starts[j + 1]] for j in range(nblocks)]
    K = [max(-(-len(per_block[qb * r + s]) // P) for r in range(4))
         for s in range(qb)]
    # true max edge count per slot across the 4 quarters (descriptor skip)
    N16 = [max(len(per_block[qb * r + s]) for r in range(4))
           for s in range(qb)]
    return per_block, K, N16


def _chunk_slots(K, budget):
    """Greedily group slot indices so each chunk's tile total <= budget."""
    chunks, cur, tot = [], [], 0
    for s, k in enumerate(K):
        if cur and tot + k > budget:
            chunks.append(cur)
            cur, tot = [], 0
        cur.append(s)
        tot += k
    if cur:
        chunks.append(cur)
    return chunks


def _wrap_idx16(idx_flat):
    """Pack int indices for dma_gather: idx j -> [j%16, j//16], tiled to 128
    partitions. idx_flat length must be a multiple of 128."""
    n = len(idx_flat)
    cols = n // 16
    arr = np.zeros((16, cols), np.int16)
    arr[np.arange(n) % 16, np.arange(n) // 16] = idx_flat
    return np.tile(arr, (8, 1))


def _pad_to(arr, n, fill):
    out = np.full(n, fill, arr.dtype)
    out[:len(arr)] = arr
    return out


class _Packed:
    pass


def _bf16():
    import concourse.mybir as mybir
    return np.dtype(mybir.dt.np(mybir.dt.bfloat16))


def _host_prep(inputs):
    f32 = np.float32
    bf16 = _bf16()
    x = np.asarray(inputs["x"], f32)
    e2h = np.asarray(inputs["e2h_idx"]).astype(np.int64)
    h2h = np.asarray(inputs["h2h_idx"]).astype(np.int64)
    h2e = np.asarray(inputs["h2e_idx"]).astype(np.int64)
    e2h_attr = np.asarray(inputs["e2h_attr"], f32)
    h2h_attr = np.asarray(inputs["h2h_attr"], f32)
    h2e_attr = np.asarray(inputs["h2e_attr"], f32)
    era_ll = np.asarray(inputs["era_latlons"], f32)
    h_ll = np.asarray(inputs["h_latlons"], f32)
    fm_ctx = np.asarray(inputs["fm_ctx"], f32)
    fm_Wsrc = np.asarray(inputs["fm_Wsrc"], f32)
    fm_Wctx = np.asarray(inputs["fm_Wctx"], f32)
    fm_Wedge = np.asarray(inputs["fm_Wedge"], f32)
    fm_att = np.asarray(inputs["fm_att"], f32)
    fm_Wval = np.asarray(inputs["fm_Wval"], f32)
    bm_ctx = np.asarray(inputs["bm_ctx"], f32)
    bm_Wsrc = np.asarray(inputs["bm_Wsrc"], f32)
    bm_Wctx = np.asarray(inputs["bm_Wctx"], f32)
    bm_Wedge = np.asarray(inputs["bm_Wedge"], f32)
    bm_att = np.asarray(inputs["bm_att"], f32)
    bm_Wval = np.asarray(inputs["bm_Wval"], f32)
    gat_W = np.asarray(inputs["gat_W"], f32)
    gat_We = np.asarray(inputs["gat_We"], f32)
    gat_asrc = np.asarray(inputs["gat_asrc"], f32)
    gat_adst = np.asarray(inputs["gat_adst"], f32)
    gat_aedge = np.asarray(inputs["gat_aedge"], f32)

    pk = _Packed()

    # ---- encoder (stage A): host computes exact per-edge alpha ----
    sA, dA = e2h[0], e2h[1]
    x_in = [np.concatenate([x[g].reshape(ERA, IN + AUX), era_ll], 1)
            for g in range(BS)]                                   # (35718,102)
    fm_w_att = fm_Wsrc @ fm_att                                   # (102,)
    uC_A = np.concatenate([fm_ctx, h_ll], 1) @ (fm_Wctx @ fm_att)  # (HMESH,)
    uE_A = e2h_attr @ (fm_Wedge @ fm_att)                         # (E,)
    alphas_A = []
    for g in range(BS):
        uS = x_in[g] @ fm_w_att                                   # (ERA,)
        logit = uS[sA] + uC_A[dA] + uE_A
        lrelu = np.where(logit >= 0, logit, 0.2 * logit)
        alphas_A.append(_seg_softmax_host(lrelu, dA, HMESH))

    pbA, KA, NA16 = _block_partition(sA, dA, NBM, QBM)

    # unique used src rows per quarter (shared across both batch groups)
    U = []
    for r in range(4):
        e = np.concatenate([pbA[QBM * r + s] for s in range(QBM)])
        U.append(np.unique(sA[e]))
    NBU = max(-(-len(u) // P) for u in U)

    # ---- processor (stage B) ----
    sB, dB = h2h[0], h2h[1]
    pbB, KB_, NB16_ = _block_partition(sB, dB, NBM, QBM)
    # split each block's edges by source half-of-quarter (chunked AG overlap)
    halfB = (sB % 2688) >= 1408
    pbB0 = [e[~halfB[e]] for e in pbB]
    pbB1 = [e[halfB[e]] for e in pbB]
    KB0 = [max(-(-len(pbB0[QBM * r + s]) // P) for r in range(4))
           for s in range(QBM)]
    KB1 = [max(-(-len(pbB1[QBM * r + s]) // P) for r in range(4))
           for s in range(QBM)]
    NB0 = [max(len(pbB0[QBM * r + s]) for r in range(4)) for s in range(QBM)]
    NB1 = [max(len(pbB1[QBM * r + s]) for r in range(4)) for s in range(QBM)]
    KB = [KB0[s] + KB1[s] for s in range(QBM)]
    pk.KB0, pk.KB1, pk.NB0, pk.NB1 = KB0, KB1, NB0, NB1
    uE_B = [h2h_attr @ np.einsum("fhd,hd->fh", gat_We[l], gat_aedge[l])
            for l in range(2)]                                    # (E,2)
    w_d = [np.einsum("fhd,hd->fh", gat_W[l], gat_adst[l]) for l in range(2)]
    # fold weights: q(256) + ud(2)
    pk.w_tb = [np.concatenate(
        [gat_W[l].reshape(HID, HID), w_d[l]], 1).astype(bf16) for l in range(2)]
    # per-edge uS = q . a_src, computed on device from gathered q.
    # physically replicated so the batched multiply needs no broadcast AP
    KMB_ = max(KB)
    pk.a_bc = [np.tile(gat_asrc[l].reshape(1, HID), (P, KMB_)).astype(bf16)
               for l in range(2)]                                 # (128,KMB*256)

    # ---- decoder (stage C) ----
    sC, dC = h2e[0], h2e[1]
    pbC, KC, NC16 = _block_partition(sC, dC, NBE, QBE)
    bm_w_att = bm_Wsrc @ bm_att                                   # (260,)
    uC_C = np.concatenate([bm_ctx, era_ll], 1) @ (bm_Wctx @ bm_att)  # (ERA,)
    uE_C = h2e_attr @ (bm_Wedge @ bm_att)                         # (E,)
    uCE_C = uC_C[dC] + uE_C

    pk.w_tc = np.concatenate(
        [bm_Wval[:HID], bm_w_att[:HID, None]], 1).astype(bf16)    # (256,97)
    hl_term = h_ll @ np.concatenate(
        [bm_Wval[HID:], bm_w_att[HID:, None]], 1)                 # (HMESH,97)
    hl_pad = np.zeros((MH_PAD, IN + 1), f32)
    hl_pad[:HMESH] = hl_term
    # per-core bias stream, partition-major: hl[p, s*97+c] = hl_pad[r*2688+s*128+p, c]
    pk.hl = []
    for r in range(4):
        q = hl_pad[QROWS * r:QROWS * (r + 1)].reshape(QBM, P, IN + 1)
        pk.hl.append(np.ascontiguousarray(
            q.transpose(1, 0, 2).reshape(P, QBM * (IN + 1))).astype(bf16))

    pk.w_ta = fm_Wval.astype(bf16)                                # (102,256)

    pk.KA, pk.KB, pk.KC = KA, KB, KC
    pk.NA16, pk.NC16 = NA16, NC16
    SKA, SKB, SKC = sum(KA), sum(KB), sum(KC)
    pk.SKA, pk.SKB, pk.SKC = SKA, SKB, SKC
    pk.NBU = NBU
    pk.KMA, pk.KMB, pk.KMC = max(KA), max(KB), max(KC)
    # stage-C slot groups (shared gather per group, <= 8 tiles)
    grpC, cur, tot = [], [], 0
    for s in range(QBE):
        if cur and tot + KC[s] > 8:
            grpC.append(cur)
            cur, tot = [], 0
        cur.append(s)
        tot += KC[s]
    if cur:
        grpC.append(cur)
    pk.grpC = grpC
    last_of_group = set(g[-1] for g in grpC)
    pk.KMAX = max(pk.KMA, pk.KMB, pk.KMC)

    # iota row template replicated for up to KMAX tiles
    iota_t = np.tile(np.arange(P, dtype=f32), pk.KMAX)
    pk.iota_t = np.tile(iota_t[None, :], (P, 1)).astype(bf16)     # (128,KMAX*128)

    def pack_quarter_A(r, g):
        sidx, cidx, alph = [], [], []
        remap = np.full(ERA, 0, np.int64)
        remap[U[r]] = np.arange(len(U[r]))
        for s in range(QBM):
            j = QBM * r + s
            e = pbA[j]
            n = KA[s] * P
            si = np.full(n, -1, np.int16)
            si[:NA16[s]] = 0
            si[:len(e)] = remap[sA[e]].astype(np.int16)
            sidx.append(si)
            cidx.append(_pad_to((dA[e] - j * P).astype(f32), n, -1.0))
            alph.append(_pad_to(alphas_A[g][e].astype(f32), n, 0.0))
        out = _Packed()
        out.sidx = _wrap_idx16(np.concatenate(sidx))
        out.cidx = np.concatenate(cidx).reshape(SKA, P).T.copy()
        out.alpha = np.concatenate(alph).reshape(SKA, P).T.copy()
        return out

    def pack_quarter_BC(r, per_block, K, qb, src, dst, SK, streams, cbc,
                        nmax, tail_neg=None):
        """streams: list of per-edge arrays (E,) or (E,m) -> packed (128, SK*m)."""
        sidx, cidx, st_out = [], [], [[] for _ in streams]
        for s in range(qb):
            j = qb * r + s
            e = per_block[j]
            n = K[s] * P
            if tail_neg is None or s in tail_neg:
                si = np.full(n, -1, np.int16)
                si[:nmax[s]] = 0
            else:
                si = np.zeros(n, np.int16)
            si[:len(e)] = src[e].astype(np.int16)
            sidx.append(si)
            cidx.append(_pad_to((dst[e] - j * P).astype(f32), n, -1.0))
            for q, arr in enumerate(streams):
                a = arr[e]
                if a.ndim == 1:
                    a = a[:, None]
                m = a.shape[1]
                buf = np.zeros((n, m), f32)
                buf[:len(e)] = a
                st_out[q].append(buf)
        out = _Packed()
        out.sidx = _wrap_idx16(np.concatenate(sidx))
        cf = np.concatenate(cidx).reshape(SK, P).T
        out.cidx = cf.copy()
        if cbc:
            # broadcast cidx: [p, t*128+c] = cidx of edge (t, c), same all p
            out.cbc = np.ascontiguousarray(
                np.tile(cf.T.reshape(1, SK * P), (P, 1))).astype(bf16)
        out.streams = []
        for q, parts in enumerate(st_out):
            a = np.concatenate(parts, 0)                          # (SK*P, m)
            m = a.shape[1]
            out.streams.append(
                a.reshape(SK, P, m).transpose(1, 0, 2).reshape(P, SK * m).copy())
        return out

    def pack_B(r):
        sidx0, sidx1, cidx, ues = [], [], [], [[], []]
        for s in range(QBM):
            j = QBM * r + s
            for half, (pb, Kh, Nh, sil, rows, off) in enumerate(
                    [(pbB0, KB0, NB0, sidx0, 1408, 0),
                     (pbB1, KB1, NB1, sidx1, 1280, 1408)]):
                e = pb[j]
                n = Kh[s] * P
                if n == 0:
                    continue
                m = sB[e]
                rem = (m // 2688) * rows + (m % 2688) - off
                si = np.full(n, -1, np.int16)
                si[:Nh[s]] = 0
                si[:len(e)] = rem.astype(np.int16)
                sil.append(si)
                cidx.append(_pad_to((dB[e] - j * P).astype(f32), n, -1.0))
                for l in range(2):
                    buf = np.zeros((n, 2), f32)
                    buf[:len(e)] = uE_B[l][e]
                    ues[l].append(buf)
        out = _Packed()
        out.sidx0 = _wrap_idx16(np.concatenate(sidx0))
        out.sidx1 = _wrap_idx16(np.concatenate(sidx1))
        cf = np.concatenate(cidx).reshape(SKB, P).T
        out.cidx = cf.copy()
        out.cbc = np.ascontiguousarray(
            np.tile(cf.T.reshape(1, SKB * P), (P, 1))).astype(bf16)
        out.streams = []
        for l in range(2):
            a = np.concatenate(ues[l], 0)
            out.streams.append(
                a.reshape(SKB, P, 2).transpose(1, 0, 2).reshape(
                    P, SKB * 2).copy())
        return out

    pk.cores = []
    for c in range(8):
        g, r = c // 4, c % 4
        pc = _Packed()
        pc.A = pack_quarter_A(r, g)
        pc.B = pack_B(r)
        pc.C = pack_quarter_BC(r, pbC, KC, QBE, sC, dC, SKC,
                               [uCE_C], cbc=False, nmax=NC16,
                               tail_neg=last_of_group)
        pc.hl = pk.hl[r]
        # compact transposed encoder input, feature-major: (102, NBU*128)
        xt = np.zeros((IN + AUX + POS, NBU * P), f32)
        xt[:, :len(U[r])] = x_in[g][U[r]].T
        pc.xinT = xt.astype(bf16)
        pk.cores.append(pc)
    return pk


# ---------------- device program ----------------

def _build(pk):
    import concourse.mybir as mybir
    import concourse.tile as tile
    from concourse import bacc
    from concourse.masks import make_identity

    f32 = mybir.dt.float32
    bf16 = mybir.dt.bfloat16
    i16 = mybir.dt.int16
    AO = mybir.AluOpType
    AF = mybir.ActivationFunctionType

    nc = bacc.Bacc("TRN2", target_bir_lowering=False, debug=False,
                   num_devices=8, num_swdge_queues=2)

    SKA, SKB, SKC = pk.SKA, pk.SKB, pk.SKC
    KA, KB, KC = pk.KA, pk.KB, pk.KC
    NA16, NC16 = pk.NA16, pk.NC16
    KB0, KB1, NB0, NB1 = pk.KB0, pk.KB1, pk.NB0, pk.NB1
    NBU, KMAX = pk.NBU, pk.KMAX
    KMA, KMB, KMC = pk.KMA, pk.KMB, pk.KMC
    IN_F = IN + AUX + POS
    ein = {}

    def xin(name, shape, dt=bf16):
        ein[name] = nc.dram_tensor(name, shape, dt, kind="ExternalInput")
        return ein[name]

    xinT = xin("xinT", [IN_F, NBU * P])
    w_ta = xin("w_ta", [IN_F, TA_W])
    w_tb0 = xin("w_tb0", [HID, HID + 2])
    w_tb1 = xin("w_tb1", [HID, HID + 2])
    a_bc0 = xin("a_bc0", [P, KMB * HID])
    a_bc1 = xin("a_bc1", [P, KMB * HID])
    w_tc = xin("w_tc", [HID, IN + 1])
    hl = xin("hl", [P, QBM * (IN + 1)])
    iota_in = xin("iota_t", [P, KMAX * P])
    a_sidx = xin("a_sidx", [P, SKA * 8], i16)
    a_cidx = xin("a_cidx", [P, SKA], f32)
    a_alpha = xin("a_alpha", [P, SKA], f32)
    SKB0, SKB1 = sum(pk.KB0), sum(pk.KB1)
    b_sidx0 = xin("b_sidx0", [P, SKB0 * 8], i16)
    b_sidx1 = xin("b_sidx1", [P, SKB1 * 8], i16)
    b_cidx = xin("b_cidx", [P, SKB], f32)
    b_cbc = xin("b_cbc", [P, SKB * P])
    b_ue0 = xin("b_ue0", [P, SKB * 2], f32)
    b_ue1 = xin("b_ue1", [P, SKB * 2], f32)
    c_sidx = xin("c_sidx", [P, SKC * 8], i16)
    c_cidx = xin("c_cidx", [P, SKC], f32)
    c_uce = xin("c_uce", [P, SKC], f32)
    out_t = nc.dram_tensor("out", [P, QBE * IN], bf16, kind="ExternalOutput")
    import os
    _lvl = int(os.environ.get("KERNEL_PHASES", "8"))
    _sub = int(os.environ.get("KERNEL_SUB", "4"))

    ofs_a = np.cumsum([0] + KA)
    ofs_b = np.cumsum([0] + KB)
    ofs_c = np.cumsum([0] + KC)

    with tile.TileContext(nc) as tc:
        with tc.tile_pool(name="const", bufs=1) as cpool, \
             tc.tile_pool(name="stream", bufs=1) as spool, \
             tc.tile_pool(name="res", bufs=1) as rpool, \
             tc.tile_pool(name="gat", bufs=3) as gpool, \
             tc.tile_pool(name="lx", bufs=2) as lxpool, \
             tc.tile_pool(name="work", bufs=3) as wpool, \
             tc.tile_pool(name="ob", bufs=2) as obpool, \
             tc.tile_pool(name="psA", bufs=2, space="PSUM") as psA, \
             tc.tile_pool(name="psU", bufs=2, space="PSUM") as psU, \
             tc.tile_pool(name="psT", bufs=2, space="PSUM") as psT, \
             tc.tile_pool(name="psF", bufs=2, space="PSUM") as psF, \
             tc.tile_pool(name="dram", bufs=1, space="DRAM") as dpool:

            # ---------- constants / streams ----------
            ident = cpool.tile([P, P], bf16, name="ident")
            make_identity(nc, ident[:])

            def load(name, src, shape, dt=bf16):
                t = spool.tile(shape, dt, name=name)
                nc.sync.dma_start(out=t[:], in_=src[tuple(slice(0, s) for s in shape)])
                return t

            w_ta_sb = load("w_ta_sb", w_ta, [IN_F, TA_W])

            def load_half(name, src, h, cols):
                t = spool.tile([P, cols], bf16, name=name)
                nc.sync.dma_start(out=t[:], in_=src[h * P:(h + 1) * P, 0:cols])
                return t[:]

            w_tb_sb = [[load_half(f"w_tb{l}_{h}", [w_tb0, w_tb1][l], h, HID + 2)
                        for h in range(2)] for l in range(2)]
            a_bc_sb = [load(f"a_bc{l}_sb", [a_bc0, a_bc1][l], [P, KMB * HID])
                       for l in range(2)]
            w_tc_sb = [load_half(f"w_tc_{h}", w_tc, h, IN + 1)
                       for h in range(2)]
            hl_sb = load("hl_sb", hl, [P, QBM * (IN + 1)])
            iota_t = load("iota_sb", iota_in, [P, KMAX * P])
            asid_sb = load("asid_sb", a_sidx, [P, SKA * 8], i16)
            acid_sb = load("acid_sb", a_cidx, [P, SKA], f32)
            aal_sb = load("aal_sb", a_alpha, [P, SKA], f32)
            bsid0_sb = load("bsid0_sb", b_sidx0, [P, SKB0 * 8], i16)
            bsid1_sb = load("bsid1_sb", b_sidx1, [P, SKB1 * 8], i16)
            bcid_sb = load("bcid_sb", b_cidx, [P, SKB], f32)
            bcbc_sb = load("bcbc_sb", b_cbc, [P, SKB * P])
            bue_sb = [load("bue0_sb", b_ue0, [P, SKB * 2], f32),
                      load("bue1_sb", b_ue1, [P, SKB * 2], f32)]
            csid_sb = load("csid_sb", c_sidx, [P, SKC * 8], i16)
            ccid_sb = load("ccid_sb", c_cidx, [P, SKC], f32)
            cuce_sb = load("cuce_sb", c_uce, [P, SKC], f32)

            # partition-index column (value = partition id), bf16
            piota_i = cpool.tile([P, 1], mybir.dt.int32, name="piota_i")
            nc.gpsimd.iota(piota_i[:], pattern=[[1, 1]], base=0,
                           channel_multiplier=1)
            piota = cpool.tile([P, 1], bf16, name="piota")
            nc.vector.tensor_copy(piota[:], piota_i[:])

            # persistent gather buffers (manual round-robin, zeroed once
            # so descriptor-skipped pad rows stay finite)
            gbufs = [rpool.tile([P, KMA * TA_W], bf16, name=f"gbuf{i}")
                     for i in range(5)]
            for t in gbufs:
                nc.vector.memset(t[:], 0.0)
            gb_i = [0]

            def next_gb():
                t = gbufs[gb_i[0] % 5]
                gb_i[0] += 1
                return t

            # ---------- resident quarter features (bf16) ----------
            xlat = rpool.tile([P, QBM * HID], bf16, name="xlat")
            h1g = rpool.tile([P, QBM * HID], bf16, name="h1g")
            xproc = rpool.tile([P, QBM * HID], bf16, name="xproc")
            ud_res = [rpool.tile([P, QBM * 2], bf16, name=f"ud{l}")
                      for l in range(2)]
            osb_res = rpool.tile([P, QBE * IN], bf16, name="osb")

            # ---------- DRAM tables (bf16) ----------
            ta_dram = dpool.tile([NBU * P, TA_W], bf16, name="ta_dram")
            HR0, HR1 = 11 * P, 10 * P
            tb_locA = [dpool.tile([HR0, TB_W], bf16, name=f"tb_locA{l}")
                       for l in range(2)]
            tb_locB = [dpool.tile([HR1, TB_W], bf16, name=f"tb_locB{l}")
                       for l in range(2)]
            tb_halfA = [dpool.tile([4 * HR0, TB_W], bf16, name=f"tb_hA{l}")
                        for l in range(2)]
            tb_halfB = [dpool.tile([4 * HR1, TB_W], bf16, name=f"tb_hB{l}")
                        for l in range(2)]
            tc_loc = dpool.tile([QROWS, TC_W], bf16, name="tc_loc")
            tc_full = dpool.tile([MH_PAD, TC_W], bf16, name="tc_full")

            # ---------- phase 1: dense T_A (compact rows) ----------
            CW = 16                               # blocks per input chunk
            for c0 in range(0, NBU if _lvl >= 1 else 0, CW):
                cw = min(CW, NBU - c0)
                lx = lxpool.tile([IN_F, CW * P], bf16, name="lx", tag="lx")
                nc.sync.dma_start(out=lx[:, 0:cw * P],
                                  in_=xinT[:, c0 * P:(c0 + cw) * P])
                for b0 in range(0, cw, 8):
                    bw = min(8, cw - b0)
                    ta_sb = wpool.tile([P, 8 * TA_W], bf16, name="ta_sb",
                                       tag="ta_sb")
                    for k in range(bw):
                        pst = psT.tile([P, TA_W], f32, name="ps_ta", tag="pst")
                        nc.tensor.matmul(
                            out=pst[:],
                            lhsT=lx[:, (b0 + k) * P:(b0 + k + 1) * P],
                            rhs=w_ta_sb[:], start=True, stop=True)
                        nc.scalar.activation(
                            ta_sb[:, k * TA_W:(k + 1) * TA_W], pst[:], AF.Copy)
                    base = (c0 + b0) * P
                    nc.sync.dma_start(
                        out=ta_dram[base:base + bw * P, :].rearrange(
                            "(b p) w -> p b w", p=P),
                        in_=ta_sb[:, 0:bw * TA_W].rearrange(
                            "p (b w) -> p b w", w=TA_W))

            # ---------- helper: one-hot for tile k of a slot ----
            def onehot(dst_ap, cidx_col):
                # dst[p, j] = (cidx[p] == j)
                nc.vector.tensor_tensor(
                    out=dst_ap, in0=cidx_col.to_broadcast([P, P]),
                    in1=iota_t[:, 0:P], op=AO.is_equal)

            # ---------- phase 2: stage A (encoder edges) ----------
            for s in range(QBM if _lvl >= 2 else 0):
                K = KA[s]
                if K == 0:
                    continue
                t0 = ofs_a[s]
                gb = next_gb()
                for c0 in range(0, K, GMAX_T):
                    cw = min(GMAX_T, K - c0)
                    nfull = cw * P
                    nreg = min(max(NA16[s] - c0 * P, 0), nfull)
                    nc.gpsimd.dma_gather(
                        out_ap=gb[:, c0 * TA_W:(c0 + cw) * TA_W].rearrange(
                            "p (k w) -> p k w", w=TA_W),
                        in_ap=ta_dram[:, :],
                        idxs_ap=asid_sb[:, (t0 + c0) * 8:(t0 + c0 + cw) * 8],
                        num_idxs=nfull, num_idxs_reg=nreg,
                        elem_size=TA_W, single_packet=False,
                        queue_num=gb_i[0] % 2)
                if True:
                    if _sub < 2:
                        continue
                    S = obpool.tile([P, KMA * P], bf16, name="S_A", tag="ob")
                    S3 = S[:, 0:K * P].rearrange("p (k c) -> p k c", c=P)
                    nc.vector.tensor_tensor(
                        out=S3,
                        in0=acid_sb[:, t0:t0 + K].rearrange(
                            "p (k one) -> p k one", one=1).to_broadcast(
                            [P, K, P]),
                        in1=iota_t[:, 0:K * P].rearrange(
                            "p (k c) -> p k c", c=P),
                        op=AO.is_equal)
                    nc.vector.tensor_tensor(
                        out=S3, in0=S3,
                        in1=aal_sb[:, t0:t0 + K].rearrange(
                            "p (k one) -> p k one", one=1).to_broadcast(
                            [P, K, P]),
                        op=AO.mult)
                    if _sub < 3:
                        continue
                    ps = psA.tile([P, HID], f32, name="psA_t", tag="psA")
                    for k in range(K):
                        nc.tensor.matmul(
                            out=ps[:], lhsT=S[:, k * P:(k + 1) * P],
                            rhs=gb[:, k * TA_W:k * TA_W + HID],
                            start=(k == 0), stop=(k == K - 1))
                    if _sub < 4:
                        continue
                    nc.scalar.activation(
                        xlat[:, s * HID:(s + 1) * HID], ps[:], AF.Copy)

            # ---------- helper: fold resident -> table ----------
            def fold_slot(src, w_sb, wcols, s, dst_dram=None, ud_dst=None,
                          bias_sb=None, dst_half=None):
                if True:
                    pst = psT.tile([P, HID], bf16, name="ps_tr", tag="pst")
                    for h in range(2):
                        nc.tensor.transpose(
                            out=pst[:, h * P:(h + 1) * P],
                            in_=src[:, s * HID + h * P:s * HID + (h + 1) * P],
                            identity=ident[:])
                    xt = wpool.tile([P, HID], bf16, name="xt", tag="xt")
                    nc.vector.tensor_copy(xt[:], pst[:])
                    psf = psF.tile([P, wcols], f32, name="ps_f", tag="psf")
                    for h in range(2):
                        nc.tensor.matmul(out=psf[:],
                                         lhsT=xt[:, h * P:(h + 1) * P],
                                         rhs=w_sb[h][:, 0:wcols],
                                         start=(h == 0), stop=(h == 1))
                    fsb = wpool.tile([P, TB_W], bf16, name="fsb", tag="fsb")
                    if bias_sb is not None:
                        nc.vector.tensor_tensor(
                            out=fsb[:, 0:wcols], in0=psf[:, 0:wcols],
                            in1=bias_sb[:, s * wcols:(s + 1) * wcols],
                            op=AO.add)
                        nc.sync.dma_start(
                            out=dst_dram[s * P:(s + 1) * P, 0:wcols],
                            in_=fsb[:, 0:wcols])
                    else:
                        nc.scalar.activation(fsb[:, 0:HID], psf[:, 0:HID],
                                             AF.Copy)
                        da, db_ = dst_half
                        dap = (da[s * P:(s + 1) * P, 0:HID] if s < 11 else
                               db_[(s - 11) * P:(s - 10) * P, 0:HID])
                        nc.sync.dma_start(out=dap, in_=fsb[:, 0:HID])
                        nc.scalar.activation(
                            ud_dst[:, 2 * s:2 * s + 2], psf[:, HID:HID + 2],
                            AF.Copy)


            def ag_half(loc, full):
                nc.gpsimd.collective_compute(
                    "AllGather", AO.bypass, replica_groups=RG,
                    ins=[loc.opt()], outs=[full.opt()])

            if _lvl >= 3:
                fold(xlat, w_tb_sb[0], HID + 2, None, ud_dst=ud_res[0],
                     dst_half=(tb_locA[0], tb_locB[0]),
                     after10=lambda: ag_half(tb_locA[0], tb_halfA[0]))
                ag_half(tb_locB[0], tb_halfB[0])

            # ---------- phase 3/4: GAT layers ----------
            ofs_b0 = np.cumsum([0] + KB0)
            ofs_b1 = np.cumsum([0] + KB1)

            def gat_layer(l, dst_res, residual, after_slot=None):
                for s in range(QBM):
                    K = KB[s]
                    if K == 0:
                        if after_slot is not None:
                            after_slot(s)
                        continue
                    t0 = ofs_b[s]
                    gb = next_gb()
                    for tfull, Kh, Nh, ofs_h, sid in (
                            (tb_halfA[l], KB0, NB0, ofs_b0, bsid0_sb),
                            (tb_halfB[l], KB1, NB1, ofs_b1, bsid1_sb)):
                        if Kh[s] == 0:
                            continue
                        o0 = 0 if tfull is tb_halfA[l] else KB0[s]
                        h0 = ofs_h[s]
                        nc.gpsimd.dma_gather(
                            out_ap=gb[:, o0 * TB_W:
                                      (o0 + Kh[s]) * TB_W].rearrange(
                                "p (k w) -> p k w", w=TB_W),
                            in_ap=tfull[:, :],
                            idxs_ap=sid[:, h0 * 8:(h0 + Kh[s]) * 8],
                            num_idxs=Kh[s] * P, num_idxs_reg=Nh[s],
                            elem_size=TB_W, single_packet=False,
                            queue_num=gb_i[0] % 2)
                    if True:
                        gsl = gb[:, 0:K * TB_W]
                        O = obpool.tile([P, KMB * P], bf16, name="O_B",
                                        tag="ob")
                        nc.vector.tensor_tensor(
                            out=O[:, 0:K * P].rearrange(
                                "p (k c) -> p k c", c=P),
                            in0=bcid_sb[:, t0:t0 + K].rearrange(
                                "p (k one) -> p k one", one=1).to_broadcast(
                                [P, K, P]),
                            in1=iota_t[:, 0:K * P].rearrange(
                                "p (k c) -> p k c", c=P),
                            op=AO.is_equal)
                        OT = wpool.tile([P, KMB * P], bf16, name="OT",
                                        tag="ot")
                        nc.vector.tensor_tensor(
                            out=OT[:, 0:K * P],
                            in0=piota[:].to_broadcast([P, K * P]),
                            in1=bcbc_sb[:, t0 * P:(t0 + K) * P],
                            op=AO.is_equal)
                        # us[p, 2k+h] = sum_d q[p,k,h,d] * a_src[h,d]
                        tmp = wpool.tile([P, KMB * TB_W], bf16, name="tmp",
                                         tag="tmp")
                        nc.vector.tensor_tensor(
                            out=tmp[:, 0:K * TB_W],
                            in0=gsl,
                            in1=a_bc_sb[l][:, 0:K * TB_W], op=AO.mult)
                        us = wpool.tile([P, 2 * KMAX], f32, name="us",
                                        tag="us")
                        nc.vector.tensor_reduce(
                            out=us[:, 0:2 * K],
                            in_=tmp[:, 0:K * TB_W].rearrange(
                                "p (k h d) -> p k h d", h=2, d=DH),
                            axis=mybir.AxisListType.X, op=AO.add)
                        psu = psU.tile([P, 2 * KMAX], f32, name="psu",
                                       tag="psu")
                        for k in range(K):
                            nc.tensor.matmul(
                                out=psu[:, 2 * k:2 * k + 2],
                                lhsT=OT[:, k * P:(k + 1) * P],
                                rhs=ud_res[l][:, 2 * s:2 * s + 2],
                                start=True, stop=True)
                        t2 = wpool.tile([P, 2 * KMAX], f32, name="t2",
                                        tag="eu")
                        nc.vector.tensor_tensor(
                            out=t2[:, 0:2 * K], in0=us[:, 0:2 * K],
                            in1=psu[:, 0:2 * K], op=AO.add)
                        t3 = wpool.tile([P, 2 * KMAX], f32, name="t3",
                                        tag="eu")
                        nc.vector.tensor_tensor(
                            out=t3[:, 0:2 * K], in0=t2[:, 0:2 * K],
                            in1=bue_sb[l][:, t0 * 2:(t0 + K) * 2], op=AO.add)
                        t4 = wpool.tile([P, 2 * KMAX], f32, name="t4",
                                        tag="eu")
                        nc.vector.scalar_tensor_tensor(
                            out=t4[:, 0:2 * K], in0=t3[:, 0:2 * K],
                            scalar=0.2, in1=t3[:, 0:2 * K],
                            op0=AO.mult, op1=AO.max)
                        eu = wpool.tile([P, 2 * KMAX], f32, name="eu",
                                        tag="eu")
                        nc.scalar.activation(eu[:, 0:2 * K], t4[:, 0:2 * K],
                                             AF.Exp)
                        # vs[p, k*258 + h*128+d] = q * eu ; cols 256:258 = eu
                        vs = wpool.tile([P, KMB * (TB_W + 2)], bf16,
                                        name="vs", tag="vs")
                        vsv = vs[:, 0:K * (TB_W + 2)].rearrange(
                            "p (k w) -> p k w", w=TB_W + 2)
                        nc.vector.tensor_tensor(
                            out=vsv[:, :, 0:TB_W].rearrange(
                                "p k (h d) -> p k h d", h=2),
                            in0=gsl.rearrange("p (k h d) -> p k h d",
                                              h=2, d=DH),
                            in1=eu[:, 0:2 * K].rearrange(
                                "p (k h one) -> p k h one",
                                h=2, one=1).to_broadcast([P, K, 2, DH]),
                            op=AO.mult)
                        nc.scalar.activation(
                            vsv[:, :, TB_W:TB_W + 2],
                            eu[:, 0:2 * K].rearrange("p (k h) -> p k h", h=2),
                            AF.Copy)
                        ps = psA.tile([P, TB_W + 2], f32, name="psB_t",
                                      tag="psA")
                        for k in range(K):
                            nc.tensor.matmul(
                                out=ps[:],
                                lhsT=O[:, k * P:(k + 1) * P],
                                rhs=vs[:, k * (TB_W + 2):(k + 1) * (TB_W + 2)],
                                start=(k == 0), stop=(k == K - 1))
                        den = wpool.tile([P, 2], f32, name="den", tag="den")
                        nc.vector.tensor_scalar_add(den[:], ps[:, TB_W:TB_W + 2],
                                                    1e-9)
                        rcp = wpool.tile([P, 2], f32, name="rcp", tag="den")
                        nc.vector.reciprocal(rcp[:], den[:])
                        for h in range(2):
                            od = dst_res[:, s * HID + h * P:s * HID + (h + 1) * P]
                            if residual is None:
                                nc.scalar.activation(
                                    od, ps[:, h * P:(h + 1) * P], AF.Copy,
                                    scale=rcp[:, h:h + 1])
                            else:
                                nc.vector.scalar_tensor_tensor(
                                    out=od, in0=ps[:, h * P:(h + 1) * P],
                                    scalar=rcp[:, h:h + 1],
                                    in1=residual[:, s * HID + h * P:
                                                 s * HID + (h + 1) * P],
                                    op0=AO.mult, op1=AO.add)
                    if after_slot is not None:
                        after_slot(s)

            # layer-1 prep emitted in halves inside gat0: gelu+fold1+AG
            def after_gat0(s):
                if _lvl < 5 or (s != 10 and s != QBM - 1):
                    return
                lo = 0 if s == 10 else 11
                for s2 in range(lo, s + 1):
                    xs = h1g[:, s2 * HID:(s2 + 1) * HID]
                    nc.scalar.activation(xs, xs, AF.Gelu_apprx_tanh)
                for s2 in range(lo, s + 1):
                    fold_slot(h1g, w_tb_sb[1], HID + 2, s2,
                              ud_dst=ud_res[1],
                              dst_half=(tb_locA[1], tb_locB[1]))
                if s == 10:
                    ag_half(tb_locA[1], tb_halfA[1])
                else:
                    ag_half(tb_locB[1], tb_halfB[1])

            if _lvl >= 4:
                gat_layer(0, h1g, None, after_slot=after_gat0)

            def after_gat1(s):
                if _lvl >= 7:
                    fold_slot(xproc, w_tc_sb, IN + 1, s, dst_dram=tc_loc,
                              bias_sb=hl_sb)

            if _lvl >= 6:
                gat_layer(1, xproc, xlat, after_slot=after_gat1)

            if _lvl >= 7:
                nc.gpsimd.collective_compute(
                    "AllGather", AO.bypass, replica_groups=RG,
                    ins=[tc_loc.opt()], outs=[tc_full.opt()])

            # ---------- phase 5: decoder (stage C) ----------
            grpC = pk.grpC
            flushC = 0
            for grp in (grpC if _lvl >= 8 else []):
                t0 = ofs_c[grp[0]]
                KT = ofs_c[grp[-1] + 1] - t0
                nreg = (ofs_c[grp[-1]] - t0) * P + NC16[grp[-1]]
                gb = next_gb()
                nc.gpsimd.dma_gather(
                    out_ap=gb[:, 0:KT * TC_W].rearrange(
                        "p (k w) -> p k w", w=TC_W),
                    in_ap=tc_full[:, :],
                    idxs_ap=csid_sb[:, t0 * 8:(t0 + KT) * 8],
                    num_idxs=KT * P, num_idxs_reg=nreg,
                    elem_size=TC_W, single_packet=False,
                    queue_num=gb_i[0] % 2)
                gsl = gb[:, 0:KT * TC_W]
                tt = wpool.tile([P, GMAX_T], f32, name="ttC", tag="eu")
                nc.vector.tensor_tensor(
                    out=tt[:, 0:KT],
                    in0=gsl.rearrange("p (k w) -> p k w",
                                      w=TC_W)[:, :, IN],
                    in1=cuce_sb[:, t0:t0 + KT], op=AO.add)
                t4 = wpool.tile([P, GMAX_T], f32, name="t4C", tag="eu")
                nc.vector.scalar_tensor_tensor(
                    out=t4[:, 0:KT], in0=tt[:, 0:KT], scalar=0.2,
                    in1=tt[:, 0:KT], op0=AO.mult, op1=AO.max)
                eu = wpool.tile([P, GMAX_T], f32, name="euC", tag="eu")
                nc.scalar.activation(eu[:, 0:KT], t4[:, 0:KT], AF.Exp)
                O = obpool.tile([P, GMAX_T * P], bf16, name="O_C", tag="ob")
                vs = wpool.tile([P, GMAX_T * (IN + 1)], bf16, name="vsC",
                                tag="vs")
                nc.vector.tensor_tensor(
                    out=O[:, 0:KT * P].rearrange("p (k c) -> p k c", c=P),
                    in0=ccid_sb[:, t0:t0 + KT].rearrange(
                        "p (k one) -> p k one", one=1).to_broadcast(
                        [P, KT, P]),
                    in1=iota_t[:, 0:KT * P].rearrange("p (k c) -> p k c", c=P),
                    op=AO.is_equal)
                vsv = vs[:, 0:KT * (IN + 1)].rearrange(
                    "p (k w) -> p k w", w=IN + 1)
                nc.vector.tensor_tensor(
                    out=vsv[:, :, 0:IN],
                    in0=gsl.rearrange("p (k w) -> p k w", w=TC_W)[:, :, 0:IN],
                    in1=eu[:, 0:KT].rearrange(
                        "p (k one) -> p k one", one=1).to_broadcast(
                        [P, KT, IN]),
                    op=AO.mult)
                nc.scalar.activation(vsv[:, :, IN], eu[:, 0:KT], AF.Copy)
                for s in grp:
                    K = KC[s]
                    b0 = ofs_c[s] - t0
                    ps = psA.tile([P, IN + 1], f32, name="psC_t", tag="psA")
                    for k in range(b0, b0 + K):
                        nc.tensor.matmul(
                            out=ps[:], lhsT=O[:, k * P:(k + 1) * P],
                            rhs=vs[:, k * (IN + 1):(k + 1) * (IN + 1)],
                            start=(k == b0), stop=(k == b0 + K - 1))
                    den = wpool.tile([P, 1], f32, name="denC", tag="den")
                    nc.vector.tensor_scalar_add(den[:], ps[:, IN:IN + 1], 1e-9)
                    rcp = wpool.tile([P, 1], f32, name="rcpC", tag="den")
                    nc.vector.reciprocal(rcp[:], den[:])
                    nc.scalar.activation(
                        osb_res[:, s * IN:(s + 1) * IN], ps[:, 0:IN],
                        AF.Copy, scale=rcp[:, 0:1])
                s = grp[-1]
                # flush finished slots in batches of ~10
                if s + 1 - flushC >= 10 or grp is grpC[-1]:
                    nc.sync.dma_start(
                        out=out_t[:, flushC * IN:(s + 1) * IN],
                        in_=osb_res[:, flushC * IN:(s + 1) * IN])
                    flushC = s + 1

    nc.compile()
    return nc


# ---------------- entry point ----------------

def _make_in_maps(pk):
    in_maps = []
    for c in range(8):
        pc = pk.cores[c]
        g, r = c // 4, c % 4
        m = {
            "xinT": pc.xinT,
            "w_ta": pk.w_ta,
            "w_tb0": pk.w_tb[0], "w_tb1": pk.w_tb[1],
            "a_bc0": pk.a_bc[0], "a_bc1": pk.a_bc[1],
            "w_tc": pk.w_tc,
            "hl": pc.hl,
            "iota_t": pk.iota_t,
            "a_sidx": pc.A.sidx, "a_cidx": pc.A.cidx, "a_alpha": pc.A.alpha,
            "b_sidx0": pc.B.sidx0, "b_sidx1": pc.B.sidx1,
            "b_cidx": pc.B.cidx, "b_cbc": pc.B.cbc,
            "b_ue0": pc.B.streams[0], "b_ue1": pc.B.streams[1],
            "c_sidx": pc.C.sidx, "c_cidx": pc.C.cidx,
            "c_uce": pc.C.streams[0],
        }
        in_maps.append({k: np.ascontiguousarray(v) for k, v in m.items()})
    return in_maps


def kernel(**inputs):
    from concourse.bass_utils import run_bass_kernel_spmd

    pk = _host_prep(inputs)
    nc = _build(pk)
    in_maps = _make_in_maps(pk)
    res = run_bass_kernel_spmd(nc, in_maps, core_ids=list(range(8)))

    x = np.asarray(inputs["x"], np.float32)
    out = np.zeros((BS, ERA, IN), np.float32)
    for g in range(BS):
        quarter = []
        for r in range(4):
            o = np.asarray(res.results[g * 4 + r]["out"]).astype(np.float32)
            quarter.append(o.reshape(P, QBE, IN).transpose(1, 0, 2).reshape(
                QBE * P, IN))
        full = np.concatenate(quarter, 0)[:ERA]
        out[g] = full + x[g, :, :IN]
    return out
